# revision 1
# baseline (speedup 1.0000x reference)
"""Trainium2 Bass kernel for nn_BDH_1726576853700 (sparse_attention).

3-layer sparse-attention net: B=1, T=1024, D=256, NH=4, N=8192, VOCAB=256.

Sharding over 8 NeuronCores: device d -> (head h=d//2, half=d%2) — each device
owns a 4096-wide slice of one head's sparse latent dim.  Within the slice the
latent index is permuted evens-first so the RoPE pair partner sits exactly 2048
rows away (tile i <-> tile i+16), turning the pair rotation into whole-tile
elementwise ops.  Per layer:
  - x_sparse^T = relu(enc_w^T @ x^T)   (local)
  - qr = rope(x_sparse)                (local, host-precomputed cos/sin tables)
  - S_partial = qr^T qr (local n contraction), strictly-causal masked
  - ykv_partial = S_masked^T @ x ; pair AllReduce (the two halves of one head)
  - ykv_ln = layernorm(ykv); y_sparse^T = relu(encv_w^T @ ykv_ln^T) (local)
  - ymlp^T_partial = dec^T-contracted with (x_sparse * y_sparse)    (local)
  - 8-way AllReduce(ymlp); x = ln(x + ln(ymlp)) (replicated)
Collectives run in fp16 (halves wire bytes); matmuls run in fp16 with fp32
PSUM accumulation; the residual stream x is kept in fp32 on-chip.

PSUM discipline: every accumulation group owns its bank(s) exclusively —
`start=True` clears has_written bits for the WHOLE bank, so two interleaved
groups must never share a bank.
"""

import math
import sys

for _p in ("/opt/trn_rl_repo",):
    if _p not in sys.path:
        sys.path.insert(0, _p)

import numpy as np

import concourse.bass as bass
import concourse.mybir as mybir
import concourse.tile as tile
from concourse import bacc, bass_utils
from concourse.masks import make_identity

# ---- problem constants (hardcoded per contract) ----
B, T, D, NH, N = 1, 1024, 256, 4, 8192
VOCAB = 256
N_LAYER = 3
EPS = 1e-5
TWO_PI = 2.0 * math.pi
N_CORES = 8
NLOC = N // 2          # latent columns per device: 4096
P = 128
NT = T // P            # 8 t-tiles
KD = D // P            # 2 d-tiles
NM = NLOC // P         # 32 n-tiles per device
NPAIR = NM // 2        # 16 rope pairs
HDT = mybir.dt.float16     # on-chip activation dtype
F32 = mybir.dt.float32
YKV_SCALE = 1.0 / 256.0    # keeps ykv in fp16 range; LN downstream is
                           # scale-invariant so the result is unchanged

_CACHE = {}


def _build_program(dbg=False, use_collectives=True, rope_gpsimd=False, skip_scores=False, skip_proj=False, n_layers=N_LAYER):
    def emit_allreduce(nc, groups, ins, outs):
        if use_collectives:
            nc.gpsimd.collective_compute(
                "AllReduce", mybir.AluOpType.add, replica_groups=groups,
                ins=ins, outs=outs)
        else:
            # timing/sim variant: replace the collective with a plain copy
            nc.sync.dma_start(outs[0], ins[0])
    nc = bacc.Bacc("TRN2", target_bir_lowering=False, debug=False,
                   num_devices=N_CORES)
    dbg_tensors = {}
    if dbg:
        for nm, shape, dt in [
            ("dbg_x0ln", [T, D], F32),
            ("dbg_xsp", [NM * P, T], HDT),
            ("dbg_qr", [NM * P, T], HDT),
            ("dbg_ykvpre", [T, D], HDT),
            ("dbg_ykvpost", [T, D], HDT),
            ("dbg_ykvT", [D, T], HDT),
            ("dbg_ymlppre", [D, T], HDT),
            ("dbg_ymlppost", [D, T], HDT),
            ("dbg_x1", [T, D], F32),
        ]:
            dbg_tensors[nm] = nc.dram_tensor(nm, shape, dt,
                                             kind="ExternalOutput")

    x0_d = nc.dram_tensor("x0", [T, D], F32, kind="ExternalInput")
    encw_d = nc.dram_tensor("encw", [D, NLOC], HDT, kind="ExternalInput")
    encvw_d = nc.dram_tensor("encvw", [D, NLOC], HDT, kind="ExternalInput")
    decw_d = nc.dram_tensor("decw", [NLOC, D], HDT, kind="ExternalInput")
    ct_d = nc.dram_tensor("ct", [NLOC // 2, T], HDT, kind="ExternalInput")
    st_d = nc.dram_tensor("st", [NLOC // 2, T], HDT, kind="ExternalInput")
    lmh_d = nc.dram_tensor("lmh", [D, VOCAB], HDT, kind="ExternalInput")
    umask_d = nc.dram_tensor("umask", [P, P], F32, kind="ExternalInput")
    logits_d = nc.dram_tensor("logits", [T, VOCAB], F32, kind="ExternalOutput")

    PAIR_GROUPS = [[0, 1], [2, 3], [4, 5], [6, 7]]
    ALL_GROUP = [list(range(N_CORES))]

    with tile.TileContext(nc) as tc:
        persist = tc.alloc_tile_pool(name="persist", bufs=1)
        dram = tc.alloc_tile_pool(name="dram", bufs=1, space="DRAM")

        # persistent SBUF state
        x_sp = persist.tile([P, NM, T], HDT)        # x_sparse^T tiles
        qr = persist.tile([P, NM, T], HDT)          # roped x_sparse^T
        x_f32 = persist.tile([P, NT, D], F32)       # residual stream (natural)
        x_h = persist.tile([P, NT, D], HDT)         # x natural fp16
        xT_h = persist.tile([P, KD, T], HDT)        # x^T fp16
        ykvT_h = persist.tile([P, KD, T], HDT)      # ykv_ln^T fp16
        lmh_sb = persist.tile([P, KD, VOCAB], HDT)
        umask_sb = persist.tile([P, P], F32)
        ident = persist.tile([P, P], HDT)

        eps_sb = persist.tile([P, 1], F32)
        nc.vector.memset(eps_sb[:], float(EPS))
        nc.sync.dma_start(umask_sb[:], umask_d.ap())
        make_identity(nc, ident[:])
        for k in range(KD):
            nc.sync.dma_start(lmh_sb[:, k, :], lmh_d.ap()[k * P:(k + 1) * P, :])

        # streaming / working pools (live across the whole kernel)
        wenc = tc.alloc_tile_pool(name="wenc", bufs=3)
        wdec = tc.alloc_tile_pool(name="wdec", bufs=4)
        csp = tc.alloc_tile_pool(name="csp", bufs=2)
        ropep = tc.alloc_tile_pool(name="ropep", bufs=2)
        schp = tc.alloc_tile_pool(name="schp", bufs=2)
        sdp = tc.alloc_tile_pool(name="sdp", bufs=2)
        yxp = tc.alloc_tile_pool(name="yxp", bufs=2)
        arp = tc.alloc_tile_pool(name="arp", bufs=1)
        lnp = tc.alloc_tile_pool(name="lnp", bufs=2)
        statp = tc.alloc_tile_pool(name="statp", bufs=4)

        def layer_norm(src_ap, out_ap):
            """LayerNorm over the free dim (size D) of a [P, D] tile."""
            stats = statp.tile([P, 6], F32, name="ln_stats")
            mv = statp.tile([P, 2], F32, name="ln_mv")
            rstd = statp.tile([P, 1], F32, name="ln_rstd")
            nc.vector.bn_stats(out=stats[:], in_=src_ap)
            nc.vector.bn_aggr(out=mv[:], in_=stats[:])
            nc.scalar.activation(out=rstd[:], in_=mv[:, 1:2],
                                 func=mybir.ActivationFunctionType.Sqrt,
                                 bias=eps_sb[:])
            nc.vector.reciprocal(out=rstd[:], in_=rstd[:])
            nc.vector.tensor_scalar(out=out_ap, in0=src_ap,
                                    scalar1=mv[:, 0:1], scalar2=rstd[:],
                                    op0=mybir.AluOpType.subtract,
                                    op1=mybir.AluOpType.mult)

        def transpose_into(dst_ap, src_ap, pst_pool):
            """PE-transpose a [P, P] fp16 SBUF block into dst (via PSUM)."""
            pst = pst_pool.tile([P, P], HDT, name="pst")
            nc.tensor.transpose(pst[:], src_ap, ident[:])
            nc.vector.tensor_copy(out=dst_ap, in_=pst[:])

        def set_x_from(j, src_f32_ap, pst_pool):
            """Write x_f32/x_h/xT_h for t-tile j from a normalized f32 tile."""
            if src_f32_ap is not x_f32:
                nc.vector.tensor_copy(out=x_f32[:, j, :], in_=src_f32_ap)
            nc.scalar.copy(out=x_h[:, j, :], in_=x_f32[:, j, :])
            for k in range(KD):
                transpose_into(xT_h[:, k, j * P:(j + 1) * P],
                               x_h[:, j, k * P:(k + 1) * P], pst_pool)

        # ---- initial x = ln(embed[idx]) (gather done on host into x0) ----
        with tc.tile_pool(name="ps_init", bufs=2, space="PSUM") as ps_init:
            for j in range(NT):
                x0t = lnp.tile([P, D], F32, name="x0t")
                nc.sync.dma_start(x0t[:], x0_d.ap()[j * P:(j + 1) * P, :])
                layer_norm(x0t[:], x_f32[:, j, :])
                set_x_from(j, x_f32, ps_init)
        if dbg:
            nc.sync.dma_start(
                dbg_tensors["dbg_x0ln"].ap().rearrange("(j p) d -> p j d", p=P),
                x_f32[:])

        # ---- layers ----
        for layer in range(n_layers):
            # Phase A: x_sparse^T = relu(enc^T x^T), then rope -> qr
            with tc.tile_pool(name=f"psA_{layer}", bufs=2,
                              space="PSUM") as psA:
                for m in range(NM):
                    ps = psA.tile([P, T], F32, name="psA")
                    et = wenc.tile([P, KD, P], HDT, name="enc_t")
                    nc.sync.dma_start(
                        et[:],
                        encw_d.ap().rearrange("(k p) n -> p k n", p=P)[
                            :, :, m * P:(m + 1) * P])
                    for c in range(2):
                        for k in range(1 if skip_proj else KD):
                            nc.tensor.matmul(
                                ps[:, c * 512:(c + 1) * 512],
                                lhsT=et[:, k, :],
                                rhs=xT_h[:, k, c * 512:(c + 1) * 512],
                                start=(k == 0),
                                stop=(k == (0 if skip_proj else KD - 1)))
                    nc.scalar.activation(
                        out=x_sp[:, m, :], in_=ps[:],
                        func=mybir.ActivationFunctionType.Relu)

                for i in range(NPAIR):
                    ctt = csp.tile([P, T], HDT, name="ctt")
                    stt = csp.tile([P, T], HDT, name="stt")
                    nc.sync.dma_start(ctt[:], ct_d.ap()[i * P:(i + 1) * P, :])
                    nc.sync.dma_start(stt[:], st_d.ap()[i * P:(i + 1) * P, :])
                    xe = x_sp[:, i, :]
                    xo = x_sp[:, i + NPAIR, :]
                    t1 = ropep.tile([P, T], HDT, name="rope_t1")
                    t2 = ropep.tile([P, T], HDT, name="rope_t2")
                    eng2 = nc.gpsimd if rope_gpsimd else nc.vector
                    nc.vector.tensor_mul(t1[:], xe, ctt[:])
                    eng2.tensor_mul(t2[:], xo, stt[:])
                    nc.vector.tensor_sub(qr[:, i, :], t1[:], t2[:])
                    t3 = ropep.tile([P, T], HDT, name="rope_t1")
                    t4 = ropep.tile([P, T], HDT, name="rope_t2")
                    nc.vector.tensor_mul(t3[:], xo, ctt[:])
                    eng2.tensor_mul(t4[:], xe, stt[:])
                    nc.vector.tensor_add(qr[:, i + NPAIR, :], t3[:], t4[:])

            if dbg and layer == 0:
                nc.sync.dma_start(
                    dbg_tensors["dbg_xsp"].ap().rearrange(
                        "(m p) t -> p m t", p=P), x_sp[:])
                nc.sync.dma_start(
                    dbg_tensors["dbg_qr"].ap().rearrange(
                        "(m p) t -> p m t", p=P), qr[:])

            # Phase B: S partial + causal mask + ykv partial accumulation.
            # c-major passes so the 4 live ykv accumulators each own a full
            # PSUM bank (plus 2 rotating banks for S chunks).
            ykv_pre = arp.tile([P, NT, D], HDT, name="ykv_pre")
            for c in range(2):
                with tc.tile_pool(name=f"psS_{layer}_{c}", bufs=3,
                                  space="PSUM") as psS, \
                     tc.tile_pool(name=f"psY_{layer}_{c}", bufs=1,
                                  space="PSUM") as psY:
                    ykv_ps = [psY.tile([P, D], F32, name=f"ykv_ps{j}",
                                       tag=f"ykv_ps{j}")
                              for j in range(4 * c, 4 * c + 4)]
                    for i in range(4 * c + 4):
                        # causal tiling: only columns t >= i*P are needed
                        base = max(c * 512, i * P)
                        width = (c + 1) * 512 - base
                        ps = psS.tile([P, 512], F32, name="psS")
                        for k in range(1 if skip_scores else NM):
                            nc.tensor.matmul(
                                ps[:, :width],
                                lhsT=qr[:, k, i * P:(i + 1) * P],
                                rhs=qr[:, k, base:base + width],
                                start=(k == 0),
                                stop=(k == (0 if skip_scores else NM - 1)))
                        sc = schp.tile([P, 512], HDT, name="schunk")
                        if i % 2 == 0:
                            nc.scalar.copy(out=sc[:, :width],
                                           in_=ps[:, :width])
                        else:
                            nc.vector.tensor_copy(out=sc[:, :width],
                                                  in_=ps[:, :width])
                        sd = None
                        if c == i // 4:
                            dcol = i * P - base
                            sd = sdp.tile([P, P], HDT, name="sdiag")
                            nc.vector.tensor_mul(sd[:],
                                                 ps[:, dcol:dcol + P],
                                                 umask_sb[:])
                        for j in range(max(4 * c, i), 4 * c + 4):
                            lhsT = sd[:] if j == i else \
                                sc[:, j * P - base:(j + 1) * P - base]
                            nc.tensor.matmul(
                                ykv_ps[j - 4 * c][:], lhsT=lhsT,
                                rhs=x_h[:, i, :],
                                start=(i == 0), stop=(i == j))
                    for j in range(4 * c, 4 * c + 4):
                        nc.scalar.mul(out=ykv_pre[:, j, :],
                                      in_=ykv_ps[j - 4 * c][:],
                                      mul=YKV_SCALE)

            if dbg and layer == 0:
                nc.sync.dma_start(
                    dbg_tensors["dbg_ykvpre"].ap().rearrange(
                        "(j p) d -> p j d", p=P), ykv_pre[:])

            # Phase C: pair AllReduce of ykv, layernorm, transpose
            ar_in = dram.tile([T, D], HDT, name=f"arin_{layer}",
                              tag=f"arin_{layer}")
            ar_out = dram.tile([T, D], HDT, name=f"arout_{layer}",
                               tag=f"arout_{layer}")
            nc.sync.dma_start(
                ar_in.rearrange("(j p) d -> p j d", p=P), ykv_pre[:])
            emit_allreduce(nc, PAIR_GROUPS, [ar_in.opt()], [ar_out.opt()])
            ykv_post = arp.tile([P, NT, D], HDT, name="ykv_post")
            nc.sync.dma_start(
                ykv_post[:], ar_out.rearrange("(j p) d -> p j d", p=P))
            with tc.tile_pool(name=f"psT_{layer}", bufs=2,
                              space="PSUM") as psT:
                for j in range(NT):
                    yl = lnp.tile([P, D], HDT, name="ykv_ln")
                    layer_norm(ykv_post[:, j, :], yl[:])
                    for k in range(KD):
                        transpose_into(ykvT_h[:, k, j * P:(j + 1) * P],
                                       yl[:, k * P:(k + 1) * P], psT)

            if dbg and layer == 0:
                nc.sync.dma_start(
                    dbg_tensors["dbg_ykvpost"].ap().rearrange(
                        "(j p) d -> p j d", p=P), ykv_post[:])
                nc.sync.dma_start(
                    dbg_tensors["dbg_ykvT"].ap().rearrange(
                        "(k p) t -> p k t", p=P), ykvT_h[:])

            # Phase D: y_sparse^T = relu(encv^T ykv_ln^T); xy = x_sp * y_sp;
            # ymlp^T accumulated transposed: lhsT = decoder tile, rhs = xy.
            # ymlp^T psum tiles span 2 banks each with exactly one
            # accumulation group per bank.
            ymlpT_pre = arp.tile([P, KD, T], HDT, name="ymlpT_pre")
            with tc.tile_pool(name=f"psD_{layer}", bufs=2,
                              space="PSUM") as psD, \
                 tc.tile_pool(name=f"psM_{layer}", bufs=1,
                              space="PSUM") as psM:
                ymlpT_ps = [psM.tile([P, T], F32, name=f"ymlpT_ps{k}",
                                     tag=f"ymlpT_ps{k}") for k in range(KD)]
                for m in range(NM):
                    ps = psD.tile([P, T], F32, name="psD")
                    et = wenc.tile([P, KD, P], HDT, name="encv_t")
                    nc.sync.dma_start(
                        et[:],
                        encvw_d.ap().rearrange("(k p) n -> p k n", p=P)[
                            :, :, m * P:(m + 1) * P])
                    for c in range(2):
                        for k in range(KD):
                            nc.tensor.matmul(
                                ps[:, c * 512:(c + 1) * 512],
                                lhsT=et[:, k, :],
                                rhs=ykvT_h[:, k, c * 512:(c + 1) * 512],
                                start=(k == 0), stop=(k == KD - 1))
                    ysp = yxp.tile([P, T], HDT, name="ysp")
                    nc.scalar.activation(
                        out=ysp[:], in_=ps[:],
                        func=mybir.ActivationFunctionType.Relu)
                    xy = yxp.tile([P, T], HDT, name="xy")
                    nc.vector.tensor_mul(xy[:], x_sp[:, m, :], ysp[:])
                    dm = wdec.tile([P, D], HDT, name="dec_t")
                    nc.sync.dma_start(dm[:],
                                      decw_d.ap()[m * P:(m + 1) * P, :])
                    for k in range(KD):
                        for c in range(2):
                            nc.tensor.matmul(
                                ymlpT_ps[k][:, c * 512:(c + 1) * 512],
                                lhsT=dm[:, k * P:(k + 1) * P],
                                rhs=xy[:, c * 512:(c + 1) * 512],
                                start=(m == 0), stop=(m == NM - 1))
                for k in range(KD):
                    nc.scalar.copy(out=ymlpT_pre[:, k, :],
                                   in_=ymlpT_ps[k][:])

            if dbg and layer == 0:
                nc.sync.dma_start(
                    dbg_tensors["dbg_ymlppre"].ap().rearrange(
                        "(k p) t -> p k t", p=P), ymlpT_pre[:])

            # Phase E: 8-way AllReduce of ymlp^T; x = ln(x + ln(ymlp))
            ar2_in = dram.tile([D, T], HDT, name=f"ar2in_{layer}",
                               tag=f"ar2in_{layer}")
            ar2_out = dram.tile([D, T], HDT, name=f"ar2out_{layer}",
                                tag=f"ar2out_{layer}", addr_space="Shared")
            nc.sync.dma_start(
                ar2_in.rearrange("(k p) t -> p k t", p=P), ymlpT_pre[:])
            emit_allreduce(nc, ALL_GROUP, [ar2_in.opt()], [ar2_out.opt()])
            ymlpT_post = arp.tile([P, KD, T], HDT, name="ymlpT_post")
            nc.sync.dma_start(
                ymlpT_post[:], ar2_out.rearrange("(k p) t -> p k t", p=P))
            if dbg and layer == 0:
                nc.sync.dma_start(
                    dbg_tensors["dbg_ymlppost"].ap().rearrange(
                        "(k p) t -> p k t", p=P), ymlpT_post[:])
            with tc.tile_pool(name=f"psE_{layer}", bufs=2,
                              space="PSUM") as psE:
                for j in range(NT):
                    ymt = lnp.tile([P, D], HDT, name="ymt")
                    for k in range(KD):
                        transpose_into(ymt[:, k * P:(k + 1) * P],
                                       ymlpT_post[:, k, j * P:(j + 1) * P],
                                       psE)
                    u = lnp.tile([P, D], F32, name="u_ln")
                    layer_norm(ymt[:], u[:])
                    xn = lnp.tile([P, D], F32, name="xn")
                    nc.vector.tensor_add(xn[:], x_f32[:, j, :], u[:])
                    layer_norm(xn[:], x_f32[:, j, :])
                    set_x_from(j, x_f32, psE)
            if dbg and layer == 0:
                nc.sync.dma_start(
                    dbg_tensors["dbg_x1"].ap().rearrange(
                        "(j p) d -> p j d", p=P), x_f32[:])

        # ---- logits = x @ lm_head ----
        with tc.tile_pool(name="psL", bufs=2, space="PSUM") as psL:
            for j in range(NT):
                ps = psL.tile([P, VOCAB], F32, name="psLt")
                for k in range(KD):
                    nc.tensor.matmul(ps[:],
                                     lhsT=xT_h[:, k, j * P:(j + 1) * P],
                                     rhs=lmh_sb[:, k, :],
                                     start=(k == 0), stop=(k == KD - 1))
                lg = lnp.tile([P, VOCAB], F32, name="lgt")
                nc.scalar.copy(out=lg[:], in_=ps[:])
                nc.sync.dma_start(logits_d.ap()[j * P:(j + 1) * P, :], lg[:])

        for _pool in (statp, lnp, arp, yxp, sdp, schp, ropep, csp,
                      wdec, wenc, dram, persist):
            _pool.release()

    nc.compile()
    return nc


def _host_inputs(idx, embed, encoder, encoder_v, decoder, lm_head):
    """Build the 8 per-core input maps (host-side sharding)."""
    f16 = np.float16
    idx = np.asarray(idx).reshape(-1).astype(np.int64)
    embed = np.asarray(embed, np.float32)
    enc = np.asarray(encoder, np.float32)
    encv = np.asarray(encoder_v, np.float32)
    dec = np.asarray(decoder, np.float32)
    lmh = np.asarray(lm_head, np.float32)

    x0 = embed[idx]  # [T, D] gather on host (pure indexing)

    # freqs exactly as the reference computes them (fp32)
    t = np.arange(0, N, dtype=np.float32)
    q = np.floor(t / 2.0) * 2.0
    freqs = (1.0 / ((2.0 ** 16) ** (q / N)) / TWO_PI).astype(np.float32)
    tvec = np.arange(T, dtype=np.float32)

    umask = (np.arange(P)[:, None] < np.arange(P)[None, :]).astype(np.float32)

    in_maps = []
    for d in range(N_CORES):
        h, half = d // 2, d % 2
        perm = np.concatenate([np.arange(0, NLOC, 2),
                               np.arange(1, NLOC, 2)]) + half * NLOC
        f_loc = freqs[perm[:NLOC // 2]]
        ph = (tvec[None, :] * f_loc[:, None]).astype(np.float32) % 1.0
        in_maps.append({
            "x0": np.ascontiguousarray(x0, np.float32),
            "encw": np.ascontiguousarray(enc[h][:, perm], f16),
            "encvw": np.ascontiguousarray(encv[h][:, perm], f16),
            "decw": np.ascontiguousarray(dec[h * N + perm, :], f16),
            "ct": np.ascontiguousarray(np.cos(TWO_PI * ph), f16),
            "st": np.ascontiguousarray(np.sin(TWO_PI * ph), f16),
            "lmh": np.ascontiguousarray(lmh, f16),
            "umask": umask,
        })
    return in_maps


def kernel(idx, embed, encoder, encoder_v, decoder, lm_head,
           _trace=False, _tmpdir=None):
    if "nc" not in _CACHE:
        _CACHE["nc"] = _build_program()
    nc = _CACHE["nc"]
    in_maps = _host_inputs(idx, embed, encoder, encoder_v, decoder, lm_head)
    res = bass_utils.run_bass_kernel_spmd(
        nc, in_maps, core_ids=list(range(N_CORES)),
        trace=_trace, tmpdir=_tmpdir)
    _CACHE["last_results"] = res
    logits = res.results[0]["logits"].astype(np.float32).reshape(B, T, VOCAB)
    return logits



# revision 42
# speedup vs baseline: 1.3886x; 1.3886x over previous
"""Trainium2 Bass kernel for nn_BDH_1726576853700 (sparse_attention).

3-layer sparse-attention net: B=1, T=1024, D=256, NH=4, N=8192, VOCAB=256.

Sharding over 8 NeuronCores: device d -> (head h=d//2, half=d%2) — each device
owns a 4096-wide slice of one head's sparse latent dim.  Within the slice the
latent index is permuted evens-first so the RoPE pair partner sits exactly 2048
rows away (k-tile (0,p) <-> (1,p)), turning the pair rotation into whole-tile
elementwise ops.  Per layer:
  - x_sparse^T = relu(enc^T @ x^T)      (local, fp16)
  - qr = rope(x_sparse) -> quantize fp8 (DVE rope, Pool fp8 convert)
  - S_partial = qr^T qr via fp8 DoubleRow matmuls (two k-tiles contracted per
    instruction = 2x PE throughput); strictly-causal upper triangle only,
    diagonal blocks masked
  - ykv_partial = S^T @ x ; pair AllReduce (the two halves of one head)
  - ykv_ln = layernorm(ykv); y_sparse^T = relu(encv^T ykv_ln^T)   (fp16)
  - ymlp^T accumulated transposed (lhsT = decoder tile), transposed back to
    natural [T, D] before the 8-way AllReduce; x = ln(x + ln(ymlp))

Perf structure: phase A (enc proj) is emitted interleaved with rope and the
first S accumulation pass (causal rows 0..2 live in 6 PSUM banks, accumulated
pair-major so S consumes qr pairs as rope emits them, 2 pairs per rope op).
Rows 3..7 run chunk-major after rope completes, interleaved with the ykv
accumulation.  LayerNorms are batched: stats for all 8 t-tiles first, one
Act Rsqrt for all 8, then in-place applies — one cross-engine round trip
instead of eight.  Weights/tables are loaded in few large DMAs, prefetched a
phase ahead on the SP queue.  Collectives run in fp16; matmuls in fp16
(scores fp8) with fp32 PSUM accumulation.

PSUM discipline: concurrent accumulation groups never share a 2KB bank
(start=True clears has_written bits for the whole bank).
"""

import math
import sys

for _p in ("/opt/trn_rl_repo",):
    if _p not in sys.path:
        sys.path.insert(0, _p)

import numpy as np

import concourse.bass as bass
import concourse.mybir as mybir
import concourse.tile as tile
from concourse import bacc, bass_utils
from concourse.masks import make_identity

# ---- problem constants (hardcoded per contract) ----
B, T, D, NH, N = 1, 1024, 256, 4, 8192
VOCAB = 256
N_LAYER = 3
EPS = 1e-5
TWO_PI = 2.0 * math.pi
N_CORES = 8
NLOC = N // 2          # latent columns per device: 4096
P = 128
NT = T // P            # 8 t-tiles
KD = D // P            # 2 d-tiles
NM = NLOC // P         # 32 n-tiles per device
NPAIR = NM // 2        # 16 rope pairs
NG = NPAIR // 2        # 8 two-pair rope groups
HDT = mybir.dt.float16     # on-chip activation dtype
F8 = mybir.dt.float8e4     # scores operand dtype (TRN e4m3, max 240)
F32 = mybir.dt.float32
YKV_SCALE = 1.0 / 256.0    # keeps ykv in fp16 range; LN downstream is
                           # scale-invariant so the result is unchanged
DR = mybir.MatmulPerfMode.DoubleRow
RELU = mybir.ActivationFunctionType.Relu
RSQRT = mybir.ActivationFunctionType.Rsqrt

# compact S storage: row-tile i holds causal cols [128*i, 1024)
S_OFF = [0]
for _i in range(1, NT):
    S_OFF.append(S_OFF[-1] + (T - P * (_i - 1)))
S_TOT = S_OFF[-1] + (T - P * (NT - 1))   # 4608

# S accumulation chunks (row, col_base, width); pass A rows 0..2 are
# pair-major (6 one-bank accumulators beside psA's 2 banks); row 3 takes
# over psA's freed banks once the enc-proj is done (pass B0, still mostly
# pair-major); rows 4..7 run chunk-major after rope completes (pass B1).
PASSA = [(0, 0, 512), (0, 512, 512), (1, 128, 384), (1, 512, 512),
         (2, 256, 256), (2, 512, 512)]
PASSB0 = [(3, 384, 128), (3, 512, 512)]
PASSB1 = [(4, 512, 512), (5, 640, 384), (6, 768, 256), (7, 896, 128)]

_CACHE = {}


def _build_program(dbg=False, use_collectives=True, n_layers=N_LAYER):
    def emit_allreduce(nc, groups, ins, outs):
        if use_collectives:
            nc.gpsimd.collective_compute(
                "AllReduce", mybir.AluOpType.add, replica_groups=groups,
                ins=ins, outs=outs)
        else:
            # timing/sim variant: replace the collective with a plain copy
            nc.sync.dma_start(outs[0], ins[0])

    nc = bacc.Bacc("TRN2", target_bir_lowering=False, debug=False,
                   num_devices=N_CORES)

    x0_d = nc.dram_tensor("x0", [T, D], HDT, kind="ExternalInput")
    encw_d = nc.dram_tensor("encw", [D, NLOC], HDT, kind="ExternalInput")
    encvw_d = nc.dram_tensor("encvw", [D, NLOC], HDT, kind="ExternalInput")
    decw_d = nc.dram_tensor("decw", [NLOC, D], HDT, kind="ExternalInput")
    ct_d = nc.dram_tensor("ct", [NLOC // 2, T], HDT, kind="ExternalInput")
    st_d = nc.dram_tensor("st", [NLOC // 2, T], HDT, kind="ExternalInput")
    lmh_d = nc.dram_tensor("lmh", [D, VOCAB], HDT, kind="ExternalInput")
    umask_d = nc.dram_tensor("umask", [P, P], F32, kind="ExternalInput")
    logits_d = nc.dram_tensor("logits", [T, VOCAB], F32, kind="ExternalOutput")

    PAIR_GROUPS = [[0, 1], [2, 3], [4, 5], [6, 7]]
    ALL_GROUP = [list(range(N_CORES))]

    with tile.TileContext(nc) as tc:
        persist = tc.alloc_tile_pool(name="persist", bufs=1)
        dram = tc.alloc_tile_pool(name="dram", bufs=1, space="DRAM")

        # persistent SBUF state
        x_h = persist.tile([P, NT, D], HDT)         # residual (natural)
        xT_h = persist.tile([P, KD, T], HDT)        # x^T fp16
        ykvT_h = persist.tile([P, KD, T], HDT)      # ykv_ln^T fp16
        x_sp = persist.tile([P, 2, NPAIR, T], HDT)  # x_sparse^T tiles
        qr8 = persist.tile([P, 2, NPAIR, T], F8)    # roped x_sparse^T, fp8
        s16 = persist.tile([P, S_TOT], HDT)         # causal S rows, compact
        lmh_sb = persist.tile([P, KD, VOCAB], HDT)
        umask_sb = persist.tile([P, P], F32)
        ident = persist.tile([P, P], HDT)
        eps_sb = persist.tile([P, 1], F32)

        # weights: one big DMA each, single-buffered (cross-layer WAR
        # rotation handled by the tile framework)
        encp = tc.alloc_tile_pool(name="encp", bufs=1)
        encvp = tc.alloc_tile_pool(name="encvp", bufs=1)
        decp = tc.alloc_tile_pool(name="decp", bufs=1)
        # rope tables: 2-pair chunks, double buffered per table
        ctp = tc.alloc_tile_pool(name="ctp", bufs=2)
        stp = tc.alloc_tile_pool(name="stp", bufs=2)
        # fp16 scratch (2 tags x bufs=2 x [P,2,T]): rope temps, ysp/xy,
        # ymlpT staging, logits staging
        wk16 = tc.alloc_tile_pool(name="wk16", bufs=2)
        # staging ring: x0 / ykv pre/post / ymlp nat pre/post share one
        # buffer, WAR-serialized through the per-layer dataflow
        stg = tc.alloc_tile_pool(name="stg", bufs=1)
        statp = tc.alloc_tile_pool(name="statp", bufs=2)

        nc.vector.memset(eps_sb[:], float(EPS))
        make_identity(nc, ident[:])

        def ln_batch(src, out_fn, n=NT):
            """Batched LayerNorm over [P, n, D] tile `src`; out_fn(j) gives
            the output AP for tile j (may alias src for in-place)."""
            statsb = statp.tile([P, NT, 6], F32, name="ln_stats")
            mvb = statp.tile([P, NT, 2], F32, name="ln_mv")
            rstdb = statp.tile([P, NT], F32, name="ln_rstd")
            for j in range(n):
                nc.vector.bn_stats(out=statsb[:, j, :], in_=src[:, j, :])
                nc.vector.bn_aggr(out=mvb[:, j, :], in_=statsb[:, j, :])
            nc.scalar.activation(out=rstdb[:, 0:n], in_=mvb[:, 0:n, 1],
                                 func=mybir.ActivationFunctionType.Sqrt,
                                 bias=eps_sb[:])
            nc.vector.reciprocal(out=rstdb[:, 0:n], in_=rstdb[:, 0:n])
            for j in range(n):
                nc.vector.tensor_scalar(out=out_fn(j), in0=src[:, j, :],
                                        scalar1=mvb[:, j, 0:1],
                                        scalar2=rstdb[:, j:j + 1],
                                        op0=mybir.AluOpType.subtract,
                                        op1=mybir.AluOpType.mult)

        def transpose_into(dst_ap, src_ap, pst_pool, copy_eng=None):
            """PE-transpose a [P, P] fp16 SBUF block into dst (via PSUM)."""
            pst = pst_pool.tile([P, P], HDT, name="pst")
            nc.tensor.transpose(pst[:], src_ap, ident[:])
            if copy_eng is nc.scalar:
                nc.scalar.copy(out=dst_ap, in_=pst[:])
            else:
                nc.vector.tensor_copy(out=dst_ap, in_=pst[:])

        def gated(t, gate):
            """WAW-dummy: delay t's load until `gate` (tiny tile) is written,
            keeping long prefetch transfers off the AR critical window.
            Touches one element of every dim-1 slice so each partial-load DMA
            picks up the ordering dependency."""
            if gate is not None:
                nc.vector.tensor_scalar_mul(out=t[:, :, 0:1], in0=t[:, :, 0:1],
                                            scalar1=gate[:, 0:1])

        def load_enc(which, gate=None):
            """Load enc/encv [P, KD, NLOC] in two DMAs (8KB contig rows)."""
            d = encw_d if which == 0 else encvw_d
            pool = encp if which == 0 else encvp
            t = pool.tile([P, KD, NLOC], HDT, name=f"w{which}")
            gated(t, gate)
            src = d.ap().rearrange("(k p) n -> p k n", p=P)
            for k in range(KD):
                nc.sync.dma_start(t[:, k, :], src[:, k, :])
            return t

        def load_dec(gate=None):
            t = decp.tile([P, NM, D], HDT, name="dec")
            gated(t, gate)
            src = decw_d.ap().rearrange("(m p) d -> p m d", p=P)
            for h in range(2):
                nc.sync.dma_start(t[:, h * 16:(h + 1) * 16, :],
                                  src[:, h * 16:(h + 1) * 16, :])
            return t

        def load_tab(g, gate=None):
            """Load rope tables for 2-pair group g: [P, 2, T] each."""
            ctt = ctp.tile([P, 2, T], HDT, name="ct")
            stt = stp.tile([P, 2, T], HDT, name="st")
            gated(ctt, gate)
            gated(stt, gate)
            src_c = ct_d.ap().rearrange("(i p) t -> p i t", p=P)
            src_s = st_d.ap().rearrange("(i p) t -> p i t", p=P)
            nc.sync.dma_start(ctt[:], src_c[:, 2 * g:2 * g + 2, :])
            nc.sync.dma_start(stt[:], src_s[:, 2 * g:2 * g + 2, :])
            return ctt, stt

        def s16_store(ps, r, base, w, eng=0):
            """Copy a PSUM S chunk into compact fp16 storage (+ diag mask).
            eng picks the copy engine (0=Act, 1=DVE) so the burst of
            end-of-pass stores spreads across idle engines (GPSIMD cannot
            read PSUM)."""
            def cp(dst, src):
                if eng == 1:
                    nc.vector.tensor_copy(out=dst, in_=src)
                else:
                    nc.scalar.copy(out=dst, in_=src)
            off = S_OFF[r] + (base - r * P)
            if base == r * P:     # chunk starts at the diagonal block
                nc.vector.tensor_mul(s16[:, off:off + P],
                                     ps[:, 0:P], umask_sb[:])
                if w > P:
                    cp(s16[:, off + P:off + w], ps[:, P:w])
            else:
                cp(s16[:, off:off + w], ps[:, :w])

        # ---- initial x = ln(embed[idx]) (gather done on host into x0) ----
        x0_sb = stg.tile([P, NT, D], HDT, name="stg")
        nc.sync.dma_start(x0_sb[:],
                          x0_d.ap().rearrange("(j p) d -> p j d", p=P))
        tabs = [load_tab(0), load_tab(1)]
        enc_sb = load_enc(0)          # layer-0 enc prefetch
        nc.sync.dma_start(umask_sb[:], umask_d.ap())
        for k in range(KD):
            nc.sync.dma_start(lmh_sb[:, k, :], lmh_d.ap()[k * P:(k + 1) * P, :])
        encv_sb = load_enc(1)
        dec_sb = load_dec()
        with tc.tile_pool(name="ps_init", bufs=2, space="PSUM") as ps_init:
            ln_batch(x0_sb, lambda j: x_h[:, j, :])
            for j in range(NT):
                for k in range(KD):
                    transpose_into(xT_h[:, k, j * P:(j + 1) * P],
                                   x_h[:, j, k * P:(k + 1) * P], ps_init,
                                   nc.scalar if (j + k) % 2 else nc.vector)

        # ---- layers ----
        for layer in range(n_layers):
            # === Phase A (enc proj + relu) / rope / S pass-A, interleaved ===
            def emit_spassA(p, psSA_tiles):
                for ci, (r, base, w) in enumerate(PASSA):
                    nc.tensor.matmul(
                        psSA_tiles[ci][:, :w],
                        lhsT=qr8[:, :, p, r * P:(r + 1) * P],
                        rhs=qr8[:, :, p, base:base + w],
                        start=(p == 0), stop=(p == NPAIR - 1),
                        perf_mode=DR)

            with tc.tile_pool(name=f"psSA_{layer}", bufs=1,
                              space="PSUM") as psSA, \
                 tc.tile_pool(name=f"psA_{layer}", bufs=2,
                              space="PSUM") as psA:
                psSA_tiles = [psSA.tile([P, w], F32, name=f"sa{ci}",
                                        tag=f"sa{ci}")
                              for ci, (r, b, w) in enumerate(PASSA)]
                for g in range(NG):
                    if g + 2 < NG:
                        tabs.append(load_tab(g + 2))
                    for mp in (2 * g, 2 * g + 1):
                        # enc proj for k-tiles (0, mp) and (1, mp)
                        for half in range(2):
                            m = half * NPAIR + mp
                            for c in range(2):
                                ps = psA.tile([P, 512], F32, name="psA")
                                for k in range(KD):
                                    nc.tensor.matmul(
                                        ps[:],
                                        lhsT=enc_sb[:, k, m * P:(m + 1) * P],
                                        rhs=xT_h[:, k,
                                                 c * 512:(c + 1) * 512],
                                        start=(k == 0), stop=(k == KD - 1))
                                dst = x_sp[:, half, mp,
                                           c * 512:(c + 1) * 512]
                                if g == 0 and (half + c) % 2 == 1:
                                    # DVE is idle before rope: split the
                                    # first group's relus to start rope ~2us
                                    # earlier
                                    nc.vector.tensor_scalar_max(
                                        out=dst, in0=ps[:], scalar1=0.0)
                                else:
                                    nc.scalar.activation(
                                        out=dst, in_=ps[:], func=RELU)
                    # rope group g: 2 pairs per DVE op, fp8 convert on Pool
                    ctt, stt = tabs[g]
                    xe = x_sp[:, 0, 2 * g:2 * g + 2, :]
                    xo = x_sp[:, 1, 2 * g:2 * g + 2, :]
                    t1 = wk16.tile([P, 2, T], HDT, name="w1")
                    t2 = wk16.tile([P, 2, T], HDT, name="w2")
                    nc.vector.tensor_mul(t1[:], xe, ctt[:])
                    nc.vector.tensor_mul(t2[:], xo, stt[:])
                    nc.vector.tensor_sub(t1[:], t1[:], t2[:])
                    nc.gpsimd.tensor_copy(out=qr8[:, 0, 2 * g:2 * g + 2, :],
                                          in_=t1[:])
                    t3 = wk16.tile([P, 2, T], HDT, name="w1")
                    t4 = wk16.tile([P, 2, T], HDT, name="w2")
                    nc.vector.tensor_mul(t3[:], xo, ctt[:])
                    nc.vector.tensor_mul(t4[:], xe, stt[:])
                    nc.vector.tensor_add(t3[:], t3[:], t4[:])
                    if g == NG - 1:
                        # last group: run the second convert on Act so both
                        # finish in parallel and S's tail starts sooner
                        nc.scalar.copy(out=qr8[:, 1, 2 * g:2 * g + 2, :],
                                       in_=t3[:])
                    else:
                        nc.gpsimd.tensor_copy(
                            out=qr8[:, 1, 2 * g:2 * g + 2, :], in_=t3[:])
                    if g >= 1:
                        emit_spassA(2 * (g - 1), psSA_tiles)
                        emit_spassA(2 * (g - 1) + 1, psSA_tiles)
                del tabs[:NG]

            # psA's banks are free now (last relu done): accumulate row 3
            # there while the rope tail finishes; pairs 0..13 are ready so
            # the PE idles less waiting for the final rope groups
            with tc.tile_pool(name=f"psB0_{layer}", bufs=1,
                              space="PSUM") as psB0:
                pb_tiles = [psB0.tile([P, w], F32, name=f"sb{ci}",
                                      tag=f"sb{ci}")
                            for ci, (r, b, w) in enumerate(PASSB0)]
                for plo, phi in ((0, NPAIR - 2), (NPAIR - 2, NPAIR)):
                    for ci, (r, base, w) in enumerate(PASSB0):
                        for p in range(plo, phi):
                            nc.tensor.matmul(
                                pb_tiles[ci][:, :w],
                                lhsT=qr8[:, :, p, r * P:(r + 1) * P],
                                rhs=qr8[:, :, p, base:base + w],
                                start=(p == 0), stop=(p == NPAIR - 1),
                                perf_mode=DR)
                emit_spassA(NPAIR - 2, psSA_tiles)
                emit_spassA(NPAIR - 1, psSA_tiles)
                # copy S chunks to compact fp16 storage (+ diag mask),
                # spread across Act/DVE
                for ci, (r, base, w) in enumerate(PASSA):
                    s16_store(psSA_tiles[ci], r, base, w, eng=ci % 2)
                for ci, (r, base, w) in enumerate(PASSB0):
                    s16_store(pb_tiles[ci], r, base, w, eng=ci % 2)

            # === S pass-B (rows 3..7, chunk-major) + ykv accumulation ===
            ykv_pre = stg.tile([P, NT, D], HDT, name="stg")
            ar_in = dram.tile([T, D], HDT, name=f"arin_{layer}",
                              tag=f"arin_{layer}")
            ar_in_v = ar_in.rearrange("(j p) d -> p j d", p=P)

            with tc.tile_pool(name=f"psSB_{layer}", bufs=3,
                              space="PSUM") as psSB, \
                 tc.tile_pool(name=f"psY_{layer}", bufs=2,
                              space="PSUM") as psY:
                def emit_ykv(j):
                    # diagonal block (i == j) last: its s16 row is the
                    # freshest, so earlier rows contract while it stores
                    ps = psY.tile([P, D], F32, name="psYt")
                    order = list(range(j)) + [j]
                    for n_, i in enumerate(order):
                        nc.tensor.matmul(
                            ps[:],
                            lhsT=s16[:, S_OFF[i] + (j - i) * P:
                                     S_OFF[i] + (j - i + 1) * P],
                            rhs=x_h[:, i, :],
                            start=(n_ == 0), stop=(n_ == j))
                    nc.scalar.mul(out=ykv_pre[:, j, :], in_=ps[:],
                                  mul=YKV_SCALE)

                first = True
                prev_r = None
                for ci, (r, base, w) in enumerate(PASSB):
                    ps = psSB.tile([P, w], F32, name="psSB")
                    for p in range(NPAIR):
                        nc.tensor.matmul(
                            ps[:],
                            lhsT=qr8[:, :, p, r * P:(r + 1) * P],
                            rhs=qr8[:, :, p, base:base + w],
                            start=(p == 0), stop=(p == NPAIR - 1),
                            perf_mode=DR)
                    s16_store(ps, r, base, w, eng=ci % 2)
                    if first:
                        # rows 0..2 were stored by pass-A: their ykv groups
                        # go right after pass-B's first chunk is in flight
                        emit_ykv(0)
                        emit_ykv(1)
                        emit_ykv(2)
                        first = False
                    if prev_r is not None and prev_r != r:
                        emit_ykv(prev_r)
                        if prev_r == 3:
                            # first ykv half done: stage it for the AllReduce
                            nc.scalar.dma_start(ar_in_v[:, 0:4, :],
                                                ykv_pre[:, 0:4, :])
                    prev_r = r
                emit_ykv(7)
                nc.scalar.dma_start(ar_in_v[:, 4:8, :], ykv_pre[:, 4:8, :])

            prefetch_next = layer + 1 < n_layers

            # === Phase C: pair AllReduce of ykv, layernorm, transpose ===
            ar_out = dram.tile([T, D], HDT, name=f"arout_{layer}",
                               tag=f"arout_{layer}")
            emit_allreduce(nc, PAIR_GROUPS, [ar_in.opt()], [ar_out.opt()])
            ykv_post = stg.tile([P, NT, D], HDT, name="stg")
            ar_out_v = ar_out.rearrange("(j p) d -> p j d", p=P)
            nc.sync.dma_start(ykv_post[:, 0:4, :], ar_out_v[:, 0:4, :])
            nc.sync.dma_start(ykv_post[:, 4:8, :], ar_out_v[:, 4:8, :])
            # next layer's tables + enc, gated past the AR window
            if prefetch_next:
                gate1 = statp.tile([P, 1], F32, name="gate")
                nc.vector.tensor_copy(out=gate1[:], in_=ykv_post[:, 7, 0:1])
                tabs = [load_tab(0, gate1), load_tab(1, gate1)]
                enc_next = load_enc(0, gate1)
            with tc.tile_pool(name=f"psT_{layer}", bufs=4,
                              space="PSUM") as psT:
                ln_batch(ykv_post, lambda j: ykv_post[:, j, :])  # in-place
                for j in range(NT):
                    for k in range(KD):
                        transpose_into(ykvT_h[:, k, j * P:(j + 1) * P],
                                       ykv_post[:, j, k * P:(k + 1) * P],
                                       psT,
                                       nc.scalar if (j + k) % 2
                                       else nc.vector)

            # === Phase D: y_sp = relu(encv^T ykv^T); xy = x_sp*y_sp;
            # ymlp^T accumulated transposed (lhsT = decoder tile).
            # c-outer so the c=0 pass starts as soon as the first half of
            # ykvT's transposes land ===
            ymlpT_k = None
            with tc.tile_pool(name=f"psD_{layer}", bufs=3,
                              space="PSUM") as psD, \
                 tc.tile_pool(name=f"psM_{layer}", bufs=1,
                              space="PSUM") as psM:
                ymlpT_ps = [psM.tile([P, T], F32, name=f"ymlpT_ps{k}",
                                     tag=f"ymlpT_ps{k}") for k in range(KD)]

                def emit_dec(m, c, xy):
                    for k in range(KD):
                        nc.tensor.matmul(
                            ymlpT_ps[k][:, c * 512:(c + 1) * 512],
                            lhsT=dec_sb[:, m, k * P:(k + 1) * P],
                            rhs=xy[:],
                            start=(m == 0), stop=(m == NM - 1))

                for c in range(2):
                    pend = []   # deferred dec matmuls (lag 2 for pipelining)
                    for m in range(NM):
                        ps = psD.tile([P, 512], F32, name="psD")
                        for k in range(KD):
                            nc.tensor.matmul(
                                ps[:],
                                lhsT=encv_sb[:, k, m * P:(m + 1) * P],
                                rhs=ykvT_h[:, k, c * 512:(c + 1) * 512],
                                start=(k == 0), stop=(k == KD - 1))
                        ysp = wk16.tile([P, 512], HDT, name="w1")
                        nc.scalar.activation(out=ysp[:], in_=ps[:],
                                             func=RELU)
                        xy = wk16.tile([P, 512], HDT, name="w2")
                        nc.vector.tensor_mul(
                            xy[:], x_sp[:, m // NPAIR, m % NPAIR,
                                        c * 512:(c + 1) * 512], ysp[:])
                        pend.append((m, c, xy))
                        if len(pend) >= 3:
                            emit_dec(*pend.pop(0))
                    for m, c_, xy in pend:
                        emit_dec(m, c_, xy)
                ymlpT_k = [wk16.tile([P, T], HDT, name="w1"),
                           wk16.tile([P, T], HDT, name="w2")]
                for k in range(KD):
                    for c in range(2):
                        eng = nc.scalar if (k + c) % 2 else nc.vector
                        src = ymlpT_ps[k][:, c * 512:(c + 1) * 512]
                        dst = ymlpT_k[k][:, c * 512:(c + 1) * 512]
                        if eng is nc.scalar:
                            nc.scalar.copy(out=dst, in_=src)
                        else:
                            nc.vector.tensor_copy(out=dst, in_=src)

            # transpose ymlp^T back to natural [T, D] BEFORE the AllReduce
            # so the post-AR critical path is just the layernorm chain;
            # stage each half for the AllReduce as soon as it is ready
            ymlp_nat = stg.tile([P, NT, D], HDT, name="stg")
            ar2_in = dram.tile([T, D], HDT, name=f"ar2in_{layer}",
                               tag=f"ar2in_{layer}")
            ar2_in_v = ar2_in.rearrange("(j p) d -> p j d", p=P)
            with tc.tile_pool(name=f"psTD_{layer}", bufs=4,
                              space="PSUM") as psTD:
                for j in range(NT):
                    for k in range(KD):
                        transpose_into(ymlp_nat[:, j, k * P:(k + 1) * P],
                                       ymlpT_k[k][:, j * P:(j + 1) * P],
                                       psTD,
                                       nc.scalar if (j + k) % 2
                                       else nc.vector)
                    if j == 3:
                        nc.scalar.dma_start(ar2_in_v[:, 0:4, :],
                                            ymlp_nat[:, 0:4, :])
                nc.scalar.dma_start(ar2_in_v[:, 4:8, :], ymlp_nat[:, 4:8, :])

            # === Phase E: 8-way AllReduce of ymlp; x = ln(x + ln(ymlp)) ===
            ar2_out = dram.tile([T, D], HDT, name=f"ar2out_{layer}",
                                tag=f"ar2out_{layer}", addr_space="Shared")
            emit_allreduce(nc, ALL_GROUP, [ar2_in.opt()], [ar2_out.opt()])
            ymlp_post = stg.tile([P, NT, D], HDT, name="stg")
            ar2_out_v = ar2_out.rearrange("(j p) d -> p j d", p=P)
            nc.sync.dma_start(ymlp_post[:, 0:4, :], ar2_out_v[:, 0:4, :])
            nc.sync.dma_start(ymlp_post[:, 4:8, :], ar2_out_v[:, 4:8, :])
            # next layer's encv/dec, gated past the AR window
            if prefetch_next:
                gate2 = statp.tile([P, 1], F32, name="gate")
                nc.vector.tensor_copy(out=gate2[:], in_=ymlp_post[:, 7, 0:1])
                encv_next = load_enc(1, gate2)
                dec_next = load_dec(gate2)
            with tc.tile_pool(name=f"psE_{layer}", bufs=4,
                              space="PSUM") as psE:
                ln_batch(ymlp_post, lambda j: ymlp_post[:, j, :])  # in-place
                for j in range(NT):
                    nc.vector.tensor_add(ymlp_post[:, j, :],
                                         ymlp_post[:, j, :], x_h[:, j, :])
                ln_batch(ymlp_post, lambda j: x_h[:, j, :])
                for j in range(NT):
                    for k in range(KD):
                        transpose_into(xT_h[:, k, j * P:(j + 1) * P],
                                       x_h[:, j, k * P:(k + 1) * P], psE,
                                       nc.scalar if (j + k) % 2
                                       else nc.vector)

            if layer + 1 < n_layers:
                enc_sb, encv_sb, dec_sb = enc_next, encv_next, dec_next

        # ---- logits = x @ lm_head ----
        with tc.tile_pool(name="psL", bufs=2, space="PSUM") as psL:
            for j in range(NT):
                ps = psL.tile([P, VOCAB], F32, name="psLt")
                for k in range(KD):
                    nc.tensor.matmul(ps[:],
                                     lhsT=xT_h[:, k, j * P:(j + 1) * P],
                                     rhs=lmh_sb[:, k, :],
                                     start=(k == 0), stop=(k == KD - 1))
                lg = wk16.tile([P, VOCAB], F32, name="w1")
                nc.scalar.copy(out=lg[:], in_=ps[:])
                nc.sync.dma_start(logits_d.ap()[j * P:(j + 1) * P, :], lg[:])

        for _pool in (statp, stg, wk16, stp, ctp, decp,
                      encvp, encp, dram, persist):
            _pool.release()

    nc.compile()
    return nc


def _host_inputs(idx, embed, encoder, encoder_v, decoder, lm_head):
    """Build the 8 per-core input maps (host-side sharding)."""
    f16 = np.float16
    idx = np.asarray(idx).reshape(-1).astype(np.int64)
    embed = np.asarray(embed, np.float32)
    enc = np.asarray(encoder, np.float32)
    encv = np.asarray(encoder_v, np.float32)
    dec = np.asarray(decoder, np.float32)
    lmh = np.asarray(lm_head, np.float32)

    x0 = embed[idx]  # [T, D] gather on host (pure indexing)

    # freqs exactly as the reference computes them (fp32)
    t = np.arange(0, N, dtype=np.float32)
    q = np.floor(t / 2.0) * 2.0
    freqs = (1.0 / ((2.0 ** 16) ** (q / N)) / TWO_PI).astype(np.float32)
    tvec = np.arange(T, dtype=np.float32)

    umask = (np.arange(P)[:, None] < np.arange(P)[None, :]).astype(np.float32)

    in_maps = []
    for d in range(N_CORES):
        h, half = d // 2, d % 2
        perm = np.concatenate([np.arange(0, NLOC, 2),
                               np.arange(1, NLOC, 2)]) + half * NLOC
        f_loc = freqs[perm[:NLOC // 2]]
        ph = (tvec[None, :] * f_loc[:, None]).astype(np.float32) % 1.0
        in_maps.append({
            "x0": np.ascontiguousarray(x0, f16),
            "encw": np.ascontiguousarray(enc[h][:, perm], f16),
            "encvw": np.ascontiguousarray(encv[h][:, perm], f16),
            "decw": np.ascontiguousarray(dec[h * N + perm, :], f16),
            "ct": np.ascontiguousarray(np.cos(TWO_PI * ph), f16),
            "st": np.ascontiguousarray(np.sin(TWO_PI * ph), f16),
            "lmh": np.ascontiguousarray(lmh, f16),
            "umask": umask,
        })
    return in_maps


def kernel(idx, embed, encoder, encoder_v, decoder, lm_head,
           _trace=False, _tmpdir=None):
    if "nc" not in _CACHE:
        _CACHE["nc"] = _build_program()
    nc = _CACHE["nc"]
    in_maps = _host_inputs(idx, embed, encoder, encoder_v, decoder, lm_head)
    res = bass_utils.run_bass_kernel_spmd(
        nc, in_maps, core_ids=list(range(N_CORES)),
        trace=_trace, tmpdir=_tmpdir)
    _CACHE["last_results"] = res
    logits = res.results[0]["logits"].astype(np.float32).reshape(B, T, VOCAB)
    return logits


# revision 50
# speedup vs baseline: 1.4128x; 1.0174x over previous
"""Trainium2 Bass kernel for nn_BDH_1726576853700 (sparse_attention).

3-layer sparse-attention net: B=1, T=1024, D=256, NH=4, N=8192, VOCAB=256.

Sharding over 8 NeuronCores: device d -> (head h=d//2, half=d%2) — each device
owns a 4096-wide slice of one head's sparse latent dim.  Within the slice the
latent index is permuted evens-first so the RoPE pair partner sits exactly 2048
rows away (k-tile (0,p) <-> (1,p)), turning the pair rotation into whole-tile
elementwise ops.  Per layer:
  - x_sparse^T = relu(enc^T @ x^T)      (local, fp16)
  - qr = rope(x_sparse) -> quantize fp8 (DVE rope, Pool fp8 convert)
  - S_partial = qr^T qr via fp8 DoubleRow matmuls (two k-tiles contracted per
    instruction = 2x PE throughput); strictly-causal upper triangle only,
    diagonal blocks masked
  - ykv_partial = S^T @ x ; pair AllReduce (the two halves of one head)
  - ykv_ln = layernorm(ykv); y_sparse^T = relu(encv^T ykv_ln^T)   (fp16)
  - ymlp^T accumulated transposed (lhsT = decoder tile), transposed back to
    natural [T, D] before the 8-way AllReduce; x = ln(x + ln(ymlp))

Perf structure: phase A (enc proj) is emitted interleaved with rope and the
first S accumulation pass (causal rows 0..2 live in 6 PSUM banks, accumulated
pair-major so S consumes qr pairs as rope emits them, 2 pairs per rope op).
Rows 3..7 run chunk-major after rope completes, interleaved with the ykv
accumulation.  LayerNorms are batched: stats for all 8 t-tiles first, one
Act Rsqrt for all 8, then in-place applies — one cross-engine round trip
instead of eight.  Weights/tables are loaded in few large DMAs, prefetched a
phase ahead on the SP queue.  Collectives run in fp16; matmuls in fp16
(scores fp8) with fp32 PSUM accumulation.

PSUM discipline: concurrent accumulation groups never share a 2KB bank
(start=True clears has_written bits for the whole bank).
"""

import math
import sys

for _p in ("/opt/trn_rl_repo",):
    if _p not in sys.path:
        sys.path.insert(0, _p)

import numpy as np

import concourse.bass as bass
import concourse.mybir as mybir
import concourse.tile as tile
from concourse import bacc, bass_utils
from concourse.masks import make_identity

# ---- problem constants (hardcoded per contract) ----
B, T, D, NH, N = 1, 1024, 256, 4, 8192
VOCAB = 256
N_LAYER = 3
EPS = 1e-5
TWO_PI = 2.0 * math.pi
N_CORES = 8
NLOC = N // 2          # latent columns per device: 4096
P = 128
NT = T // P            # 8 t-tiles
KD = D // P            # 2 d-tiles
NM = NLOC // P         # 32 n-tiles per device
NPAIR = NM // 2        # 16 rope pairs
NG = NPAIR // 2        # 8 two-pair rope groups
HDT = mybir.dt.float16     # on-chip activation dtype
F8 = mybir.dt.float8e4     # scores operand dtype (TRN e4m3, max 240)
F32 = mybir.dt.float32
YKV_SCALE = 1.0 / 256.0    # keeps ykv in fp16 range; LN downstream is
                           # scale-invariant so the result is unchanged
DR = mybir.MatmulPerfMode.DoubleRow
RELU = mybir.ActivationFunctionType.Relu
RSQRT = mybir.ActivationFunctionType.Rsqrt

# compact S storage: row-tile i holds causal cols [128*i, 1024)
S_OFF = [0]
for _i in range(1, NT):
    S_OFF.append(S_OFF[-1] + (T - P * (_i - 1)))
S_TOT = S_OFF[-1] + (T - P * (NT - 1))   # 4608

# S accumulation chunks (row, col_base, width); pass A rows 0..2 are
# pair-major (6 one-bank accumulators beside psA's 2 banks); row 3 takes
# over psA's freed banks once the enc-proj is done (pass B0, still mostly
# pair-major); rows 4..7 run chunk-major after rope completes (pass B1).
PASSA = [(0, 0, 512), (0, 512, 512), (1, 128, 384), (1, 512, 512),
         (2, 256, 256), (2, 512, 512)]
PASSB0 = [(3, 384, 128), (3, 512, 512)]
PASSB1 = [(4, 512, 512), (5, 640, 384), (6, 768, 256), (7, 896, 128)]

_CACHE = {}


def _build_program(dbg=False, use_collectives=True, n_layers=N_LAYER):
    def emit_allreduce(nc, groups, ins, outs):
        if use_collectives:
            nc.gpsimd.collective_compute(
                "AllReduce", mybir.AluOpType.add, replica_groups=groups,
                ins=ins, outs=outs)
        else:
            # timing/sim variant: replace the collective with a plain copy
            nc.sync.dma_start(outs[0], ins[0])

    nc = bacc.Bacc("TRN2", target_bir_lowering=False, debug=False,
                   num_devices=N_CORES)

    x0_d = nc.dram_tensor("x0", [T, D], HDT, kind="ExternalInput")
    encw_d = nc.dram_tensor("encw", [D, NLOC], HDT, kind="ExternalInput")
    encvw_d = nc.dram_tensor("encvw", [D, NLOC], HDT, kind="ExternalInput")
    decw_d = nc.dram_tensor("decw", [NLOC, D], HDT, kind="ExternalInput")
    ct_d = nc.dram_tensor("ct", [NLOC // 2, T], HDT, kind="ExternalInput")
    st_d = nc.dram_tensor("st", [NLOC // 2, T], HDT, kind="ExternalInput")
    lmh_d = nc.dram_tensor("lmh", [D, VOCAB], HDT, kind="ExternalInput")
    umask_d = nc.dram_tensor("umask", [P, P], F32, kind="ExternalInput")
    logits_d = nc.dram_tensor("logits", [T, VOCAB], F32, kind="ExternalOutput")

    PAIR_GROUPS = [[0, 1], [2, 3], [4, 5], [6, 7]]
    ALL_GROUP = [list(range(N_CORES))]

    with tile.TileContext(nc) as tc:
        persist = tc.alloc_tile_pool(name="persist", bufs=1)
        dram = tc.alloc_tile_pool(name="dram", bufs=1, space="DRAM")

        # persistent SBUF state
        x_h = persist.tile([P, NT, D], HDT)         # residual (natural)
        xT_h = persist.tile([P, KD, T], HDT)        # x^T fp16
        ykvT_h = persist.tile([P, KD, T], HDT)      # ykv_ln^T fp16
        x_sp = persist.tile([P, 2, NPAIR, T], HDT)  # x_sparse^T tiles
        qr8 = persist.tile([P, 2, NPAIR, T], F8)    # roped x_sparse^T, fp8
        s16 = persist.tile([P, S_TOT], HDT)         # causal S rows, compact
        lmh_sb = persist.tile([P, KD, VOCAB], HDT)
        umask_sb = persist.tile([P, P], F32)
        ident = persist.tile([P, P], HDT)
        eps_sb = persist.tile([P, 1], F32)

        # weights: one big DMA each, single-buffered (cross-layer WAR
        # rotation handled by the tile framework)
        encp = tc.alloc_tile_pool(name="encp", bufs=1)
        encvp = tc.alloc_tile_pool(name="encvp", bufs=1)
        decp = tc.alloc_tile_pool(name="decp", bufs=1)
        # rope tables: 2-pair chunks, double buffered per table
        ctp = tc.alloc_tile_pool(name="ctp", bufs=2)
        stp = tc.alloc_tile_pool(name="stp", bufs=2)
        # fp16 scratch (2 tags x bufs=2 x [P,2,T]): rope temps, ysp/xy,
        # ymlpT staging, logits staging
        wk16 = tc.alloc_tile_pool(name="wk16", bufs=2)
        # staging ring: x0 / ykv pre/post / ymlp nat pre/post share one
        # buffer, WAR-serialized through the per-layer dataflow
        stg = tc.alloc_tile_pool(name="stg", bufs=1)
        statp = tc.alloc_tile_pool(name="statp", bufs=2)

        nc.vector.memset(eps_sb[:], float(EPS))
        make_identity(nc, ident[:])

        def ln_batch(src, out_fn, n=NT):
            """Batched LayerNorm over [P, n, D] tile `src`; out_fn(j) gives
            the output AP for tile j (may alias src for in-place)."""
            statsb = statp.tile([P, NT, 6], F32, name="ln_stats")
            mvb = statp.tile([P, NT, 2], F32, name="ln_mv")
            rstdb = statp.tile([P, NT], F32, name="ln_rstd")
            for j in range(n):
                nc.vector.bn_stats(out=statsb[:, j, :], in_=src[:, j, :])
                nc.vector.bn_aggr(out=mvb[:, j, :], in_=statsb[:, j, :])
            nc.scalar.activation(out=rstdb[:, 0:n], in_=mvb[:, 0:n, 1],
                                 func=mybir.ActivationFunctionType.Sqrt,
                                 bias=eps_sb[:])
            nc.vector.reciprocal(out=rstdb[:, 0:n], in_=rstdb[:, 0:n])
            for j in range(n):
                nc.vector.tensor_scalar(out=out_fn(j), in0=src[:, j, :],
                                        scalar1=mvb[:, j, 0:1],
                                        scalar2=rstdb[:, j:j + 1],
                                        op0=mybir.AluOpType.subtract,
                                        op1=mybir.AluOpType.mult)

        def transpose_into(dst_ap, src_ap, pst_pool, copy_eng=None):
            """PE-transpose a [P, P] fp16 SBUF block into dst (via PSUM)."""
            pst = pst_pool.tile([P, P], HDT, name="pst")
            nc.tensor.transpose(pst[:], src_ap, ident[:])
            if copy_eng is nc.scalar:
                nc.scalar.copy(out=dst_ap, in_=pst[:])
            else:
                nc.vector.tensor_copy(out=dst_ap, in_=pst[:])

        def gated(t, gate):
            """WAW-dummy: delay t's load until `gate` (tiny tile) is written,
            keeping long prefetch transfers off the AR critical window.
            Touches one element of every dim-1 slice so each partial-load DMA
            picks up the ordering dependency."""
            if gate is not None:
                nc.vector.tensor_scalar_mul(out=t[:, :, 0:1], in0=t[:, :, 0:1],
                                            scalar1=gate[:, 0:1])

        def load_enc(which, gate=None):
            """Load enc/encv [P, KD, NLOC] in two DMAs (8KB contig rows)."""
            d = encw_d if which == 0 else encvw_d
            pool = encp if which == 0 else encvp
            t = pool.tile([P, KD, NLOC], HDT, name=f"w{which}")
            gated(t, gate)
            src = d.ap().rearrange("(k p) n -> p k n", p=P)
            for k in range(KD):
                nc.sync.dma_start(t[:, k, :], src[:, k, :])
            return t

        def load_dec(gate=None):
            t = decp.tile([P, NM, D], HDT, name="dec")
            gated(t, gate)
            src = decw_d.ap().rearrange("(m p) d -> p m d", p=P)
            for h in range(2):
                nc.sync.dma_start(t[:, h * 16:(h + 1) * 16, :],
                                  src[:, h * 16:(h + 1) * 16, :])
            return t

        def load_tab(g, gate=None):
            """Load rope tables for 2-pair group g: [P, 2, T] each."""
            ctt = ctp.tile([P, 2, T], HDT, name="ct")
            stt = stp.tile([P, 2, T], HDT, name="st")
            gated(ctt, gate)
            gated(stt, gate)
            src_c = ct_d.ap().rearrange("(i p) t -> p i t", p=P)
            src_s = st_d.ap().rearrange("(i p) t -> p i t", p=P)
            nc.sync.dma_start(ctt[:], src_c[:, 2 * g:2 * g + 2, :])
            nc.sync.dma_start(stt[:], src_s[:, 2 * g:2 * g + 2, :])
            return ctt, stt

        def s16_store(ps, r, base, w, eng=0):
            """Copy a PSUM S chunk into compact fp16 storage (+ diag mask).
            eng picks the copy engine (0=Act, 1=DVE) so the burst of
            end-of-pass stores spreads across idle engines (GPSIMD cannot
            read PSUM)."""
            def cp(dst, src):
                if eng == 1:
                    nc.vector.tensor_copy(out=dst, in_=src)
                else:
                    nc.scalar.copy(out=dst, in_=src)
            off = S_OFF[r] + (base - r * P)
            if base == r * P:     # chunk starts at the diagonal block
                nc.vector.tensor_mul(s16[:, off:off + P],
                                     ps[:, 0:P], umask_sb[:])
                if w > P:
                    cp(s16[:, off + P:off + w], ps[:, P:w])
            else:
                cp(s16[:, off:off + w], ps[:, :w])

        # ---- initial x = ln(embed[idx]) (gather done on host into x0) ----
        x0_sb = stg.tile([P, NT, D], HDT, name="stg")
        nc.sync.dma_start(x0_sb[:],
                          x0_d.ap().rearrange("(j p) d -> p j d", p=P))
        tabs = [load_tab(0), load_tab(1)]
        enc_sb = load_enc(0)          # layer-0 enc prefetch
        nc.sync.dma_start(umask_sb[:], umask_d.ap())
        for k in range(KD):
            nc.sync.dma_start(lmh_sb[:, k, :], lmh_d.ap()[k * P:(k + 1) * P, :])
        encv_sb = load_enc(1)
        dec_sb = load_dec()
        with tc.tile_pool(name="ps_init", bufs=2, space="PSUM") as ps_init:
            ln_batch(x0_sb, lambda j: x_h[:, j, :])
            for j in range(NT):
                for k in range(KD):
                    transpose_into(xT_h[:, k, j * P:(j + 1) * P],
                                   x_h[:, j, k * P:(k + 1) * P], ps_init,
                                   nc.scalar if (j + k) % 2 else nc.vector)

        # ---- layers ----
        for layer in range(n_layers):
            # === Phase A (enc proj + relu) / rope / S pass-A, interleaved ===
            def emit_spassA(p, psSA_tiles):
                for ci, (r, base, w) in enumerate(PASSA):
                    nc.tensor.matmul(
                        psSA_tiles[ci][:, :w],
                        lhsT=qr8[:, :, p, r * P:(r + 1) * P],
                        rhs=qr8[:, :, p, base:base + w],
                        start=(p == 0), stop=(p == NPAIR - 1),
                        perf_mode=DR)

            a_scope = tc.tile_pool(name=f"psA_{layer}", bufs=2,
                                   space="PSUM")
            with tc.tile_pool(name=f"psSA_{layer}", bufs=1,
                              space="PSUM") as psSA:
                psSA_tiles = [psSA.tile([P, w], F32, name=f"sa{ci}",
                                        tag=f"sa{ci}")
                              for ci, (r, b, w) in enumerate(PASSA)]
                psA = a_scope.__enter__()
                for g in range(NG):
                    if g + 2 < NG:
                        tabs.append(load_tab(g + 2))
                    for mp in (2 * g, 2 * g + 1):
                        # enc proj for k-tiles (0, mp) and (1, mp)
                        for half in range(2):
                            m = half * NPAIR + mp
                            for c in range(2):
                                ps = psA.tile([P, 512], F32, name="psA")
                                for k in range(KD):
                                    nc.tensor.matmul(
                                        ps[:],
                                        lhsT=enc_sb[:, k, m * P:(m + 1) * P],
                                        rhs=xT_h[:, k,
                                                 c * 512:(c + 1) * 512],
                                        start=(k == 0), stop=(k == KD - 1))
                                dst = x_sp[:, half, mp,
                                           c * 512:(c + 1) * 512]
                                if g == 0 and (half + c) % 2 == 1:
                                    # DVE is idle before rope: split the
                                    # first group's relus to start rope ~2us
                                    # earlier
                                    nc.vector.tensor_scalar_max(
                                        out=dst, in0=ps[:], scalar1=0.0)
                                else:
                                    nc.scalar.activation(
                                        out=dst, in_=ps[:], func=RELU)
                    # rope group g: 2 pairs per DVE op, fp8 convert on Pool
                    ctt, stt = tabs[g]
                    xe = x_sp[:, 0, 2 * g:2 * g + 2, :]
                    xo = x_sp[:, 1, 2 * g:2 * g + 2, :]
                    t1 = wk16.tile([P, 2, T], HDT, name="w1")
                    t2 = wk16.tile([P, 2, T], HDT, name="w2")
                    nc.vector.tensor_mul(t1[:], xe, ctt[:])
                    nc.vector.tensor_mul(t2[:], xo, stt[:])
                    nc.vector.tensor_sub(t1[:], t1[:], t2[:])
                    nc.gpsimd.tensor_copy(out=qr8[:, 0, 2 * g:2 * g + 2, :],
                                          in_=t1[:])
                    t3 = wk16.tile([P, 2, T], HDT, name="w1")
                    t4 = wk16.tile([P, 2, T], HDT, name="w2")
                    nc.vector.tensor_mul(t3[:], xo, ctt[:])
                    nc.vector.tensor_mul(t4[:], xe, stt[:])
                    nc.vector.tensor_add(t3[:], t3[:], t4[:])
                    if g == NG - 1:
                        # last group: second convert on Act (relus drained)
                        # so both converts finish in parallel and the S tail
                        # starts right at rope end
                        nc.scalar.copy(out=qr8[:, 1, 2 * g:2 * g + 2, :],
                                       in_=t3[:])
                    else:
                        nc.gpsimd.tensor_copy(
                            out=qr8[:, 1, 2 * g:2 * g + 2, :], in_=t3[:])
                    if g >= 1:
                        emit_spassA(2 * (g - 1), psSA_tiles)
                        emit_spassA(2 * (g - 1) + 1, psSA_tiles)
                del tabs[:NG]
                a_scope.__exit__(None, None, None)

                # psA's banks are free now (last relu done): accumulate row
                # 3 there while the rope tail finishes; pairs 0..13 are
                # ready so the PE idles less waiting for the last groups
                with tc.tile_pool(name=f"psB0_{layer}", bufs=1,
                                  space="PSUM") as psB0:
                    pb_tiles = [psB0.tile([P, w], F32, name=f"sb{ci}",
                                          tag=f"sb{ci}")
                                for ci, (r, b, w) in enumerate(PASSB0)]
                    for plo, phi in ((0, NPAIR - 2), (NPAIR - 2, NPAIR)):
                        for ci, (r, base, w) in enumerate(PASSB0):
                            for p in range(plo, phi):
                                nc.tensor.matmul(
                                    pb_tiles[ci][:, :w],
                                    lhsT=qr8[:, :, p, r * P:(r + 1) * P],
                                    rhs=qr8[:, :, p, base:base + w],
                                    start=(p == 0), stop=(p == NPAIR - 1),
                                    perf_mode=DR)
                    emit_spassA(NPAIR - 2, psSA_tiles)
                    emit_spassA(NPAIR - 1, psSA_tiles)
                    # copy S chunks to compact fp16 storage (+ diag mask),
                    # spread across Act/DVE
                    for ci, (r, base, w) in enumerate(PASSA):
                        s16_store(psSA_tiles[ci], r, base, w, eng=ci % 2)
                    for ci, (r, base, w) in enumerate(PASSB0):
                        s16_store(pb_tiles[ci], r, base, w, eng=ci % 2)

            # === S pass-B (rows 3..7, chunk-major) + ykv accumulation ===
            ykv_pre = stg.tile([P, NT, D], HDT, name="stg")
            ar_in = dram.tile([T, D], HDT, name=f"arin_{layer}",
                              tag=f"arin_{layer}")
            ar_in_v = ar_in.rearrange("(j p) d -> p j d", p=P)

            with tc.tile_pool(name=f"psSB_{layer}", bufs=3,
                              space="PSUM") as psSB, \
                 tc.tile_pool(name=f"psY_{layer}", bufs=2,
                              space="PSUM") as psY:
                def emit_ykv(j):
                    # diagonal block (i == j) last: its s16 row is the
                    # freshest, so earlier rows contract while it stores
                    ps = psY.tile([P, D], F32, name="psYt")
                    order = list(range(j)) + [j]
                    for n_, i in enumerate(order):
                        nc.tensor.matmul(
                            ps[:],
                            lhsT=s16[:, S_OFF[i] + (j - i) * P:
                                     S_OFF[i] + (j - i + 1) * P],
                            rhs=x_h[:, i, :],
                            start=(n_ == 0), stop=(n_ == j))
                    nc.scalar.mul(out=ykv_pre[:, j, :], in_=ps[:],
                                  mul=YKV_SCALE)

                # rows 0..3 are stored: their ykv groups + first AR half
                for j in range(4):
                    emit_ykv(j)
                nc.scalar.dma_start(ar_in_v[:, 0:4, :], ykv_pre[:, 0:4, :])
                for ci, (r, base, w) in enumerate(PASSB1):
                    ps = psSB.tile([P, w], F32, name="psSB")
                    for p in range(NPAIR):
                        nc.tensor.matmul(
                            ps[:],
                            lhsT=qr8[:, :, p, r * P:(r + 1) * P],
                            rhs=qr8[:, :, p, base:base + w],
                            start=(p == 0), stop=(p == NPAIR - 1),
                            perf_mode=DR)
                    s16_store(ps, r, base, w, eng=ci % 2)
                    emit_ykv(r)
                nc.scalar.dma_start(ar_in_v[:, 4:8, :], ykv_pre[:, 4:8, :])

            prefetch_next = layer + 1 < n_layers

            # === Phase C: pair AllReduce of ykv, layernorm, transpose ===
            ar_out = dram.tile([T, D], HDT, name=f"arout_{layer}",
                               tag=f"arout_{layer}")
            emit_allreduce(nc, PAIR_GROUPS, [ar_in.opt()], [ar_out.opt()])
            ykv_post = stg.tile([P, NT, D], HDT, name="stg")
            ar_out_v = ar_out.rearrange("(j p) d -> p j d", p=P)
            nc.sync.dma_start(ykv_post[:, 0:4, :], ar_out_v[:, 0:4, :])
            nc.sync.dma_start(ykv_post[:, 4:8, :], ar_out_v[:, 4:8, :])
            # next layer's tables + enc, gated past the AR window
            if prefetch_next:
                gate1 = statp.tile([P, 1], F32, name="gate")
                nc.vector.tensor_copy(out=gate1[:], in_=ykv_post[:, 7, 0:1])
                tabs = [load_tab(0, gate1), load_tab(1, gate1)]
                enc_next = load_enc(0, gate1)
            with tc.tile_pool(name=f"psT_{layer}", bufs=4,
                              space="PSUM") as psT:
                ln_batch(ykv_post, lambda j: ykv_post[:, j, :])  # in-place
                for j in range(NT):
                    for k in range(KD):
                        transpose_into(ykvT_h[:, k, j * P:(j + 1) * P],
                                       ykv_post[:, j, k * P:(k + 1) * P],
                                       psT,
                                       nc.scalar if (j + k) % 2
                                       else nc.vector)

            # === Phase D: y_sp = relu(encv^T ykv^T); xy = x_sp*y_sp;
            # ymlp^T accumulated transposed (lhsT = decoder tile).
            # c-outer so the c=0 pass starts as soon as the first half of
            # ykvT's transposes land ===
            # Each c-half's accumulators complete at the end of its m-pass,
            # so the c=0 epilogue (PSUM copy, transpose to natural [T, D],
            # AllReduce staging) hides under the c=1 pass's compute.
            ymlp_nat = stg.tile([P, NT, D], HDT, name="stg")
            ar2_in = dram.tile([T, D], HDT, name=f"ar2in_{layer}",
                               tag=f"ar2in_{layer}")
            ar2_in_v = ar2_in.rearrange("(j p) d -> p j d", p=P)
            with tc.tile_pool(name=f"psD_{layer}", bufs=2,
                              space="PSUM") as psD, \
                 tc.tile_pool(name=f"psM_{layer}", bufs=1,
                              space="PSUM") as psM, \
                 tc.tile_pool(name=f"psTD_{layer}", bufs=2,
                              space="PSUM") as psTD:
                ymlpT_ps = [psM.tile([P, T], F32, name=f"ymlpT_ps{k}",
                                     tag=f"ymlpT_ps{k}") for k in range(KD)]
                ymlpT_k = [wk16.tile([P, T], HDT, name="ymT"),
                           wk16.tile([P, T], HDT, name="ymT")]

                def emit_dec(m, c, xy):
                    for k in range(KD):
                        nc.tensor.matmul(
                            ymlpT_ps[k][:, c * 512:(c + 1) * 512],
                            lhsT=dec_sb[:, m, k * P:(k + 1) * P],
                            rhs=xy[:],
                            start=(m == 0), stop=(m == NM - 1))

                def epilogue_c(c):
                    """PSUM->SBUF copies, transposes, AR staging for half c"""
                    for k in range(KD):
                        src = ymlpT_ps[k][:, c * 512:(c + 1) * 512]
                        dst = ymlpT_k[k][:, c * 512:(c + 1) * 512]
                        if (k + c) % 2:
                            nc.scalar.copy(out=dst, in_=src)
                        else:
                            nc.vector.tensor_copy(out=dst, in_=src)
                    for j in range(4 * c, 4 * c + 4):
                        for k in range(KD):
                            transpose_into(ymlp_nat[:, j, k * P:(k + 1) * P],
                                           ymlpT_k[k][:, j * P:(j + 1) * P],
                                           psTD,
                                           nc.scalar if (j + k) % 2
                                           else nc.vector)
                    nc.scalar.dma_start(ar2_in_v[:, 4 * c:4 * c + 4, :],
                                        ymlp_nat[:, 4 * c:4 * c + 4, :])

                for c in range(2):
                    pend = []   # deferred dec matmuls (lag 2 for pipelining)
                    for m in range(NM):
                        ps = psD.tile([P, 512], F32, name="psD")
                        for k in range(KD):
                            nc.tensor.matmul(
                                ps[:],
                                lhsT=encv_sb[:, k, m * P:(m + 1) * P],
                                rhs=ykvT_h[:, k, c * 512:(c + 1) * 512],
                                start=(k == 0), stop=(k == KD - 1))
                        ysp = wk16.tile([P, 512], HDT, name="w1")
                        nc.scalar.activation(out=ysp[:], in_=ps[:],
                                             func=RELU)
                        xy = wk16.tile([P, 512], HDT, name="w2")
                        nc.vector.tensor_mul(
                            xy[:], x_sp[:, m // NPAIR, m % NPAIR,
                                        c * 512:(c + 1) * 512], ysp[:])
                        pend.append((m, c, xy))
                        if len(pend) >= 3:
                            emit_dec(*pend.pop(0))
                        if c == 1 and m == 8:
                            epilogue_c(0)
                    for m, c_, xy in pend:
                        emit_dec(m, c_, xy)
                epilogue_c(1)

            # === Phase E: 8-way AllReduce of ymlp; x = ln(x + ln(ymlp)) ===
            ar2_out = dram.tile([T, D], HDT, name=f"ar2out_{layer}",
                                tag=f"ar2out_{layer}", addr_space="Shared")
            emit_allreduce(nc, ALL_GROUP, [ar2_in.opt()], [ar2_out.opt()])
            ymlp_post = stg.tile([P, NT, D], HDT, name="stg")
            ar2_out_v = ar2_out.rearrange("(j p) d -> p j d", p=P)
            nc.sync.dma_start(ymlp_post[:, 0:4, :], ar2_out_v[:, 0:4, :])
            nc.sync.dma_start(ymlp_post[:, 4:8, :], ar2_out_v[:, 4:8, :])
            # next layer's encv/dec, gated past the AR window
            if prefetch_next:
                gate2 = statp.tile([P, 1], F32, name="gate")
                nc.vector.tensor_copy(out=gate2[:], in_=ymlp_post[:, 7, 0:1])
                encv_next = load_enc(1, gate2)
                dec_next = load_dec(gate2)
            last = layer == n_layers - 1
            with tc.tile_pool(name=f"psE_{layer}", bufs=4,
                              space="PSUM") as psE, \
                 tc.tile_pool(name=f"psL_{layer}", bufs=2,
                              space="PSUM") as psL:
                ln_batch(ymlp_post, lambda j: ymlp_post[:, j, :])  # in-place
                for j in range(NT):
                    nc.vector.tensor_add(ymlp_post[:, j, :],
                                         ymlp_post[:, j, :], x_h[:, j, :])
                ln_batch(ymlp_post, lambda j: x_h[:, j, :])
                for j in range(NT):
                    for k in range(KD):
                        transpose_into(xT_h[:, k, j * P:(j + 1) * P],
                                       x_h[:, j, k * P:(k + 1) * P], psE,
                                       nc.scalar if (j + k) % 2
                                       else nc.vector)
                    if last:
                        # logits = x @ lm_head, fused into the final layer's
                        # epilogue per t-tile
                        ps = psL.tile([P, VOCAB], F32, name="psLt")
                        for k in range(KD):
                            nc.tensor.matmul(
                                ps[:], lhsT=xT_h[:, k, j * P:(j + 1) * P],
                                rhs=lmh_sb[:, k, :],
                                start=(k == 0), stop=(k == KD - 1))
                        lg = wk16.tile([P, VOCAB], F32, name="w1")
                        nc.scalar.copy(out=lg[:], in_=ps[:])
                        nc.sync.dma_start(
                            logits_d.ap()[j * P:(j + 1) * P, :], lg[:])

            if layer + 1 < n_layers:
                enc_sb, encv_sb, dec_sb = enc_next, encv_next, dec_next

        for _pool in (statp, stg, wk16, stp, ctp, decp,
                      encvp, encp, dram, persist):
            _pool.release()

    nc.compile()
    return nc


def _host_inputs(idx, embed, encoder, encoder_v, decoder, lm_head):
    """Build the 8 per-core input maps (host-side sharding)."""
    f16 = np.float16
    idx = np.asarray(idx).reshape(-1).astype(np.int64)
    embed = np.asarray(embed, np.float32)
    enc = np.asarray(encoder, np.float32)
    encv = np.asarray(encoder_v, np.float32)
    dec = np.asarray(decoder, np.float32)
    lmh = np.asarray(lm_head, np.float32)

    x0 = embed[idx]  # [T, D] gather on host (pure indexing)

    # freqs exactly as the reference computes them (fp32)
    t = np.arange(0, N, dtype=np.float32)
    q = np.floor(t / 2.0) * 2.0
    freqs = (1.0 / ((2.0 ** 16) ** (q / N)) / TWO_PI).astype(np.float32)
    tvec = np.arange(T, dtype=np.float32)

    umask = (np.arange(P)[:, None] < np.arange(P)[None, :]).astype(np.float32)

    in_maps = []
    for d in range(N_CORES):
        h, half = d // 2, d % 2
        perm = np.concatenate([np.arange(0, NLOC, 2),
                               np.arange(1, NLOC, 2)]) + half * NLOC
        f_loc = freqs[perm[:NLOC // 2]]
        ph = (tvec[None, :] * f_loc[:, None]).astype(np.float32) % 1.0
        in_maps.append({
            "x0": np.ascontiguousarray(x0, f16),
            "encw": np.ascontiguousarray(enc[h][:, perm], f16),
            "encvw": np.ascontiguousarray(encv[h][:, perm], f16),
            "decw": np.ascontiguousarray(dec[h * N + perm, :], f16),
            "ct": np.ascontiguousarray(np.cos(TWO_PI * ph), f16),
            "st": np.ascontiguousarray(np.sin(TWO_PI * ph), f16),
            "lmh": np.ascontiguousarray(lmh, f16),
            "umask": umask,
        })
    return in_maps


def kernel(idx, embed, encoder, encoder_v, decoder, lm_head,
           _trace=False, _tmpdir=None):
    if "nc" not in _CACHE:
        _CACHE["nc"] = _build_program()
    nc = _CACHE["nc"]
    in_maps = _host_inputs(idx, embed, encoder, encoder_v, decoder, lm_head)
    res = bass_utils.run_bass_kernel_spmd(
        nc, in_maps, core_ids=list(range(N_CORES)),
        trace=_trace, tmpdir=_tmpdir)
    _CACHE["last_results"] = res
    logits = res.results[0]["logits"].astype(np.float32).reshape(B, T, VOCAB)
    return logits


# revision 55
# speedup vs baseline: 1.4146x; 1.0013x over previous
"""Trainium2 Bass kernel for nn_BDH_1726576853700 (sparse_attention).

3-layer sparse-attention net: B=1, T=1024, D=256, NH=4, N=8192, VOCAB=256.

Sharding over 8 NeuronCores: device d -> (head h=d//2, half=d%2) — each device
owns a 4096-wide slice of one head's sparse latent dim.  Within the slice the
latent index is permuted evens-first so the RoPE pair partner sits exactly 2048
rows away (k-tile (0,p) <-> (1,p)), turning the pair rotation into whole-tile
elementwise ops.  Per layer:
  - x_sparse^T = relu(enc^T @ x^T)      (local, fp16)
  - qr = rope(x_sparse) -> quantize fp8 (DVE rope, Pool fp8 convert)
  - S_partial = qr^T qr via fp8 DoubleRow matmuls (two k-tiles contracted per
    instruction = 2x PE throughput); strictly-causal upper triangle only,
    diagonal blocks masked
  - ykv_partial = S^T @ x ; pair AllReduce (the two halves of one head)
  - ykv_ln = layernorm(ykv); y_sparse^T = relu(encv^T ykv_ln^T)   (fp16)
  - ymlp^T accumulated transposed (lhsT = decoder tile), transposed back to
    natural [T, D] before the 8-way AllReduce; x = ln(x + ln(ymlp))

Perf structure: phase A (enc proj) is emitted interleaved with rope and the
first S accumulation pass (causal rows 0..2 live in 6 PSUM banks, accumulated
pair-major so S consumes qr pairs as rope emits them, 2 pairs per rope op).
Rows 3..7 run chunk-major after rope completes, interleaved with the ykv
accumulation.  LayerNorms are batched: stats for all 8 t-tiles first, one
Act Rsqrt for all 8, then in-place applies — one cross-engine round trip
instead of eight.  Weights/tables are loaded in few large DMAs, prefetched a
phase ahead on the SP queue.  Collectives run in fp16; matmuls in fp16
(scores fp8) with fp32 PSUM accumulation.

PSUM discipline: concurrent accumulation groups never share a 2KB bank
(start=True clears has_written bits for the whole bank).
"""

import math
import sys

for _p in ("/opt/trn_rl_repo",):
    if _p not in sys.path:
        sys.path.insert(0, _p)

import numpy as np

import concourse.bass as bass
import concourse.mybir as mybir
import concourse.tile as tile
from concourse import bacc, bass_utils
from concourse.masks import make_identity

# ---- problem constants (hardcoded per contract) ----
B, T, D, NH, N = 1, 1024, 256, 4, 8192
VOCAB = 256
N_LAYER = 3
EPS = 1e-5
TWO_PI = 2.0 * math.pi
N_CORES = 8
NLOC = N // 2          # latent columns per device: 4096
P = 128
NT = T // P            # 8 t-tiles
KD = D // P            # 2 d-tiles
NM = NLOC // P         # 32 n-tiles per device
NPAIR = NM // 2        # 16 rope pairs
NG = NPAIR // 2        # 8 two-pair rope groups
HDT = mybir.dt.float16     # on-chip activation dtype
F8 = mybir.dt.float8e4     # scores operand dtype (TRN e4m3, max 240)
F32 = mybir.dt.float32
YKV_SCALE = 1.0 / 256.0    # keeps ykv in fp16 range; LN downstream is
                           # scale-invariant so the result is unchanged
DR = mybir.MatmulPerfMode.DoubleRow
RELU = mybir.ActivationFunctionType.Relu
RSQRT = mybir.ActivationFunctionType.Rsqrt

# compact S storage: row-tile i holds causal cols [128*i, 1024)
S_OFF = [0]
for _i in range(1, NT):
    S_OFF.append(S_OFF[-1] + (T - P * (_i - 1)))
S_TOT = S_OFF[-1] + (T - P * (NT - 1))   # 4608

# S accumulation chunks (row, col_base, width); pass A rows 0..2 are
# pair-major (6 one-bank accumulators beside psA's 2 banks); row 3 takes
# over psA's freed banks once the enc-proj is done (pass B0, still mostly
# pair-major); rows 4..7 run chunk-major after rope completes (pass B1).
PASSA = [(0, 0, 512), (0, 512, 512), (1, 128, 384), (1, 512, 512),
         (2, 256, 256), (2, 512, 512)]
PASSB0 = [(3, 384, 128), (3, 512, 512)]
PASSB1 = [(4, 512, 512), (5, 640, 384), (6, 768, 256), (7, 896, 128)]

_CACHE = {}


def _build_program(dbg=False, use_collectives=True, n_layers=N_LAYER):
    def emit_allreduce(nc, groups, ins, outs):
        if use_collectives:
            nc.gpsimd.collective_compute(
                "AllReduce", mybir.AluOpType.add, replica_groups=groups,
                ins=ins, outs=outs)
        else:
            # timing/sim variant: replace the collective with a plain copy
            nc.sync.dma_start(outs[0], ins[0])

    nc = bacc.Bacc("TRN2", target_bir_lowering=False, debug=False,
                   num_devices=N_CORES)

    x0_d = nc.dram_tensor("x0", [T, D], HDT, kind="ExternalInput")
    encw_d = nc.dram_tensor("encw", [D, NLOC], HDT, kind="ExternalInput")
    encvw_d = nc.dram_tensor("encvw", [D, NLOC], HDT, kind="ExternalInput")
    decw_d = nc.dram_tensor("decw", [NLOC, D], HDT, kind="ExternalInput")
    ct_d = nc.dram_tensor("ct", [NLOC // 2, T], HDT, kind="ExternalInput")
    st_d = nc.dram_tensor("st", [NLOC // 2, T], HDT, kind="ExternalInput")
    lmh_d = nc.dram_tensor("lmh", [D, VOCAB], HDT, kind="ExternalInput")
    umask_d = nc.dram_tensor("umask", [P, P], F32, kind="ExternalInput")
    logits_d = nc.dram_tensor("logits", [T, VOCAB], F32, kind="ExternalOutput")

    PAIR_GROUPS = [[0, 1], [2, 3], [4, 5], [6, 7]]
    ALL_GROUP = [list(range(N_CORES))]

    with tile.TileContext(nc) as tc:
        persist = tc.alloc_tile_pool(name="persist", bufs=1)
        dram = tc.alloc_tile_pool(name="dram", bufs=1, space="DRAM")

        # persistent SBUF state
        x_h = persist.tile([P, NT, D], HDT)         # residual (natural)
        xT_h = persist.tile([P, KD, T], HDT)        # x^T fp16
        ykvT_h = persist.tile([P, KD, T], HDT)      # ykv_ln^T fp16
        x_sp = persist.tile([P, 2, NPAIR, T], HDT)  # x_sparse^T tiles
        qr8 = persist.tile([P, 2, NPAIR, T], F8)    # roped x_sparse^T, fp8
        s16 = persist.tile([P, S_TOT], HDT)         # causal S rows, compact
        umask_sb = persist.tile([P, P], F32)
        ident = persist.tile([P, P], HDT)
        eps_sb = persist.tile([P, 1], F32)

        # weights: one big DMA each, single-buffered (cross-layer WAR
        # rotation handled by the tile framework)
        encp = tc.alloc_tile_pool(name="encp", bufs=1)
        encvp = tc.alloc_tile_pool(name="encvp", bufs=1)
        decp = tc.alloc_tile_pool(name="decp", bufs=1)
        # rope tables: 2-pair chunks, double buffered per table
        ctp = tc.alloc_tile_pool(name="ctp", bufs=2)
        stp = tc.alloc_tile_pool(name="stp", bufs=2)
        # fp16 scratch (2 tags x bufs=2 x [P,2,T]): rope temps, ysp/xy,
        # ymlpT staging, logits staging
        wk16 = tc.alloc_tile_pool(name="wk16", bufs=2)
        # staging ring: x0 / ykv pre/post / ymlp nat pre/post share one
        # buffer, WAR-serialized through the per-layer dataflow
        stg = tc.alloc_tile_pool(name="stg", bufs=1)
        statp = tc.alloc_tile_pool(name="statp", bufs=2)

        nc.vector.memset(eps_sb[:], float(EPS))
        make_identity(nc, ident[:])

        def ln_batch(src, out_fn, lo=0, hi=NT):
            """Batched LayerNorm over tiles lo..hi of a [P, NT, D] tile
            `src`; out_fn(j) gives the output AP for tile j (may alias src
            for in-place).  Caller can process halves so consumers of the
            first tiles start before the second half's stats."""
            statsb = statp.tile([P, NT, 6], F32, name="ln_stats")
            mvb = statp.tile([P, NT, 2], F32, name="ln_mv")
            rstdb = statp.tile([P, NT], F32, name="ln_rstd")
            for j in range(lo, hi):
                nc.vector.bn_stats(out=statsb[:, j, :], in_=src[:, j, :])
                nc.vector.bn_aggr(out=mvb[:, j, :], in_=statsb[:, j, :])
            nc.scalar.activation(out=rstdb[:, lo:hi], in_=mvb[:, lo:hi, 1],
                                 func=mybir.ActivationFunctionType.Sqrt,
                                 bias=eps_sb[:])
            nc.vector.reciprocal(out=rstdb[:, lo:hi], in_=rstdb[:, lo:hi])
            for j in range(lo, hi):
                nc.vector.tensor_scalar(out=out_fn(j), in0=src[:, j, :],
                                        scalar1=mvb[:, j, 0:1],
                                        scalar2=rstdb[:, j:j + 1],
                                        op0=mybir.AluOpType.subtract,
                                        op1=mybir.AluOpType.mult)

        def transpose_into(dst_ap, src_ap, pst_pool, copy_eng=None):
            """PE-transpose a [P, P] fp16 SBUF block into dst (via PSUM)."""
            pst = pst_pool.tile([P, P], HDT, name="pst")
            nc.tensor.transpose(pst[:], src_ap, ident[:])
            if copy_eng is nc.scalar:
                nc.scalar.copy(out=dst_ap, in_=pst[:])
            else:
                nc.vector.tensor_copy(out=dst_ap, in_=pst[:])

        def gated(t, gate):
            """WAW-dummy: delay t's load until `gate` (tiny tile) is written,
            keeping long prefetch transfers off the AR critical window.
            Touches one element of every dim-1 slice so each partial-load DMA
            picks up the ordering dependency."""
            if gate is not None:
                nc.vector.tensor_scalar_mul(out=t[:, :, 0:1], in0=t[:, :, 0:1],
                                            scalar1=gate[:, 0:1])

        def load_enc(which, gate=None):
            """Load enc/encv [P, KD, NLOC] in two DMAs (8KB contig rows)."""
            d = encw_d if which == 0 else encvw_d
            pool = encp if which == 0 else encvp
            t = pool.tile([P, KD, NLOC], HDT, name=f"w{which}")
            gated(t, gate)
            src = d.ap().rearrange("(k p) n -> p k n", p=P)
            for k in range(KD):
                nc.sync.dma_start(t[:, k, :], src[:, k, :])
            return t

        def load_dec(gate=None):
            t = decp.tile([P, NM, D], HDT, name="dec")
            gated(t, gate)
            src = decw_d.ap().rearrange("(m p) d -> p m d", p=P)
            for h in range(2):
                nc.sync.dma_start(t[:, h * 16:(h + 1) * 16, :],
                                  src[:, h * 16:(h + 1) * 16, :])
            return t

        def load_tab(g, gate=None):
            """Load rope tables for 2-pair group g: [P, 2, T] each."""
            ctt = ctp.tile([P, 2, T], HDT, name="ct")
            stt = stp.tile([P, 2, T], HDT, name="st")
            gated(ctt, gate)
            gated(stt, gate)
            src_c = ct_d.ap().rearrange("(i p) t -> p i t", p=P)
            src_s = st_d.ap().rearrange("(i p) t -> p i t", p=P)
            nc.sync.dma_start(ctt[:], src_c[:, 2 * g:2 * g + 2, :])
            nc.sync.dma_start(stt[:], src_s[:, 2 * g:2 * g + 2, :])
            return ctt, stt

        def s16_store(ps, r, base, w, eng=0):
            """Copy a PSUM S chunk into compact fp16 storage (+ diag mask).
            eng picks the copy engine (0=Act, 1=DVE) so the burst of
            end-of-pass stores spreads across idle engines (GPSIMD cannot
            read PSUM)."""
            def cp(dst, src):
                if eng == 1:
                    nc.vector.tensor_copy(out=dst, in_=src)
                else:
                    nc.scalar.copy(out=dst, in_=src)
            off = S_OFF[r] + (base - r * P)
            if base == r * P:     # chunk starts at the diagonal block
                nc.vector.tensor_mul(s16[:, off:off + P],
                                     ps[:, 0:P], umask_sb[:])
                if w > P:
                    cp(s16[:, off + P:off + w], ps[:, P:w])
            else:
                cp(s16[:, off:off + w], ps[:, :w])

        # ---- initial x = ln(embed[idx]) (gather done on host into x0) ----
        x0_sb = stg.tile([P, NT, D], HDT, name="stg")
        nc.sync.dma_start(x0_sb[:],
                          x0_d.ap().rearrange("(j p) d -> p j d", p=P))
        tabs = [load_tab(0), load_tab(1)]
        enc_sb = load_enc(0)          # layer-0 enc prefetch
        nc.sync.dma_start(umask_sb[:], umask_d.ap())
        encv_sb = load_enc(1)
        dec_sb = load_dec()
        with tc.tile_pool(name="ps_init", bufs=2, space="PSUM") as ps_init:
            ln_batch(x0_sb, lambda j: x_h[:, j, :])
            for j in range(NT):
                for k in range(KD):
                    transpose_into(xT_h[:, k, j * P:(j + 1) * P],
                                   x_h[:, j, k * P:(k + 1) * P], ps_init,
                                   nc.scalar if (j + k) % 2 else nc.vector)

        # ---- layers ----
        for layer in range(n_layers):
            # === Phase A (enc proj + relu) / rope / S pass-A, interleaved ===
            def emit_spassA(p, psSA_tiles):
                for ci, (r, base, w) in enumerate(PASSA):
                    nc.tensor.matmul(
                        psSA_tiles[ci][:, :w],
                        lhsT=qr8[:, :, p, r * P:(r + 1) * P],
                        rhs=qr8[:, :, p, base:base + w],
                        start=(p == 0), stop=(p == NPAIR - 1),
                        perf_mode=DR)

            a_scope = tc.tile_pool(name=f"psA_{layer}", bufs=2,
                                   space="PSUM")
            with tc.tile_pool(name=f"psSA_{layer}", bufs=1,
                              space="PSUM") as psSA:
                psSA_tiles = [psSA.tile([P, w], F32, name=f"sa{ci}",
                                        tag=f"sa{ci}")
                              for ci, (r, b, w) in enumerate(PASSA)]
                psA = a_scope.__enter__()
                for g in range(NG):
                    if g + 2 < NG:
                        tabs.append(load_tab(g + 2))
                    for mp in (2 * g, 2 * g + 1):
                        # enc proj for k-tiles (0, mp) and (1, mp)
                        for half in range(2):
                            m = half * NPAIR + mp
                            for c in range(2):
                                ps = psA.tile([P, 512], F32, name="psA")
                                for k in range(KD):
                                    nc.tensor.matmul(
                                        ps[:],
                                        lhsT=enc_sb[:, k, m * P:(m + 1) * P],
                                        rhs=xT_h[:, k,
                                                 c * 512:(c + 1) * 512],
                                        start=(k == 0), stop=(k == KD - 1))
                                dst = x_sp[:, half, mp,
                                           c * 512:(c + 1) * 512]
                                if g == 0 and (half + c) % 2 == 1:
                                    # DVE is idle before rope: split the
                                    # first group's relus to start rope ~2us
                                    # earlier
                                    nc.vector.tensor_scalar_max(
                                        out=dst, in0=ps[:], scalar1=0.0)
                                else:
                                    nc.scalar.activation(
                                        out=dst, in_=ps[:], func=RELU)
                    # rope group g: 2 pairs per DVE op, fp8 convert on Pool
                    ctt, stt = tabs[g]
                    xe = x_sp[:, 0, 2 * g:2 * g + 2, :]
                    xo = x_sp[:, 1, 2 * g:2 * g + 2, :]
                    # mul temps (w1/w2) are freed by the DVE itself (sub/add
                    # read them); the conv input lives in its own tag (w3) so
                    # a slow Pool convert never stalls the rope mul stream
                    t1 = wk16.tile([P, 2, T], HDT, name="w1")
                    t2 = wk16.tile([P, 2, T], HDT, name="w2", bufs=1)
                    nc.vector.tensor_mul(t1[:], xe, ctt[:])
                    nc.vector.tensor_mul(t2[:], xo, stt[:])
                    for h in range(2):
                        qe = wk16.tile([P, T], HDT, name="w3")
                        nc.vector.tensor_sub(qe[:], t1[:, h, :], t2[:, h, :])
                        nc.gpsimd.tensor_copy(out=qr8[:, 0, 2 * g + h, :],
                                              in_=qe[:])
                    t3 = wk16.tile([P, 2, T], HDT, name="w1")
                    t4 = wk16.tile([P, 2, T], HDT, name="w2", bufs=1)
                    nc.vector.tensor_mul(t3[:], xo, ctt[:])
                    nc.vector.tensor_mul(t4[:], xe, stt[:])
                    for h in range(2):
                        qo = wk16.tile([P, T], HDT, name="w3")
                        nc.vector.tensor_add(qo[:], t3[:, h, :], t4[:, h, :])
                        if g == NG - 1:
                            # last group: converts on Act (relus drained) so
                            # the S tail starts right at rope end
                            nc.scalar.copy(out=qr8[:, 1, 2 * g + h, :],
                                           in_=qo[:])
                        else:
                            nc.gpsimd.tensor_copy(
                                out=qr8[:, 1, 2 * g + h, :], in_=qo[:])
                    if g >= 1:
                        emit_spassA(2 * (g - 1), psSA_tiles)
                        emit_spassA(2 * (g - 1) + 1, psSA_tiles)
                del tabs[:NG]
                a_scope.__exit__(None, None, None)

                # psA's banks are free now (last relu done): accumulate row
                # 3 there while the rope tail finishes; pairs 0..13 are
                # ready so the PE idles less waiting for the last groups
                with tc.tile_pool(name=f"psB0_{layer}", bufs=1,
                                  space="PSUM") as psB0:
                    pb_tiles = [psB0.tile([P, w], F32, name=f"sb{ci}",
                                          tag=f"sb{ci}")
                                for ci, (r, b, w) in enumerate(PASSB0)]
                    for plo, phi in ((0, NPAIR - 2), (NPAIR - 2, NPAIR)):
                        for ci, (r, base, w) in enumerate(PASSB0):
                            for p in range(plo, phi):
                                nc.tensor.matmul(
                                    pb_tiles[ci][:, :w],
                                    lhsT=qr8[:, :, p, r * P:(r + 1) * P],
                                    rhs=qr8[:, :, p, base:base + w],
                                    start=(p == 0), stop=(p == NPAIR - 1),
                                    perf_mode=DR)
                    emit_spassA(NPAIR - 2, psSA_tiles)
                    emit_spassA(NPAIR - 1, psSA_tiles)
                    # copy S chunks to compact fp16 storage (+ diag mask),
                    # spread across Act/DVE
                    for ci, (r, base, w) in enumerate(PASSA):
                        s16_store(psSA_tiles[ci], r, base, w, eng=ci % 2)
                    for ci, (r, base, w) in enumerate(PASSB0):
                        s16_store(pb_tiles[ci], r, base, w, eng=ci % 2)

            # === S pass-B (rows 3..7, chunk-major) + ykv accumulation ===
            ykv_pre = stg.tile([P, NT, D], HDT, name="stg")
            ar_in = dram.tile([T, D], HDT, name=f"arin_{layer}",
                              tag=f"arin_{layer}")
            ar_in_v = ar_in.rearrange("(j p) d -> p j d", p=P)

            with tc.tile_pool(name=f"psSB_{layer}", bufs=3,
                              space="PSUM") as psSB, \
                 tc.tile_pool(name=f"psY_{layer}", bufs=2,
                              space="PSUM") as psY:
                def emit_ykv(j):
                    # diagonal block (i == j) last: its s16 row is the
                    # freshest, so earlier rows contract while it stores
                    ps = psY.tile([P, D], F32, name="psYt")
                    order = list(range(j)) + [j]
                    for n_, i in enumerate(order):
                        nc.tensor.matmul(
                            ps[:],
                            lhsT=s16[:, S_OFF[i] + (j - i) * P:
                                     S_OFF[i] + (j - i + 1) * P],
                            rhs=x_h[:, i, :],
                            start=(n_ == 0), stop=(n_ == j))
                    nc.scalar.mul(out=ykv_pre[:, j, :], in_=ps[:],
                                  mul=YKV_SCALE)

                # rows 0..3 are stored: their ykv groups + first AR half
                for j in range(4):
                    emit_ykv(j)
                nc.scalar.dma_start(ar_in_v[:, 0:4, :], ykv_pre[:, 0:4, :])
                for ci, (r, base, w) in enumerate(PASSB1):
                    ps = psSB.tile([P, w], F32, name="psSB")
                    for p in range(NPAIR):
                        nc.tensor.matmul(
                            ps[:],
                            lhsT=qr8[:, :, p, r * P:(r + 1) * P],
                            rhs=qr8[:, :, p, base:base + w],
                            start=(p == 0), stop=(p == NPAIR - 1),
                            perf_mode=DR)
                    s16_store(ps, r, base, w, eng=ci % 2)
                    emit_ykv(r)
                nc.scalar.dma_start(ar_in_v[:, 4:8, :], ykv_pre[:, 4:8, :])

            prefetch_next = layer + 1 < n_layers

            # === Phase C: pair AllReduce of ykv, layernorm, transpose ===
            ar_out = dram.tile([T, D], HDT, name=f"arout_{layer}",
                               tag=f"arout_{layer}")
            emit_allreduce(nc, PAIR_GROUPS, [ar_in.opt()], [ar_out.opt()])
            ykv_post = stg.tile([P, NT, D], HDT, name="stg")
            ar_out_v = ar_out.rearrange("(j p) d -> p j d", p=P)
            nc.sync.dma_start(ykv_post[:, 0:4, :], ar_out_v[:, 0:4, :])
            nc.sync.dma_start(ykv_post[:, 4:8, :], ar_out_v[:, 4:8, :])
            # next layer's tables + enc, gated past the AR window
            if prefetch_next:
                gate1 = statp.tile([P, 1], F32, name="gate")
                nc.vector.tensor_copy(out=gate1[:], in_=ykv_post[:, 7, 0:1])
                tabs = [load_tab(0, gate1), load_tab(1, gate1)]
                enc_next = load_enc(0, gate1)
            with tc.tile_pool(name=f"psT_{layer}", bufs=4,
                              space="PSUM") as psT:
                for lo in (0, 4):   # halves: D's c=0 needs only tiles 0..3
                    ln_batch(ykv_post, lambda j: ykv_post[:, j, :],
                             lo, lo + 4)   # in-place
                    for j in range(lo, lo + 4):
                        for k in range(KD):
                            transpose_into(ykvT_h[:, k, j * P:(j + 1) * P],
                                           ykv_post[:, j, k * P:(k + 1) * P],
                                           psT,
                                           nc.scalar if (j + k) % 2
                                           else nc.vector)

            # === Phase D: y_sp = relu(encv^T ykv^T); xy = x_sp*y_sp;
            # ymlp^T accumulated transposed (lhsT = decoder tile).
            # c-outer so the c=0 pass starts as soon as the first half of
            # ykvT's transposes land ===
            # Each c-half's accumulators complete at the end of its m-pass,
            # so the c=0 epilogue (PSUM copy, transpose to natural [T, D],
            # AllReduce staging) hides under the c=1 pass's compute.
            ymlp_nat = stg.tile([P, NT, D], HDT, name="stg")
            ar2_in = dram.tile([T, D], HDT, name=f"ar2in_{layer}",
                               tag=f"ar2in_{layer}")
            ar2_in_v = ar2_in.rearrange("(j p) d -> p j d", p=P)
            with tc.tile_pool(name=f"psD_{layer}", bufs=2,
                              space="PSUM") as psD, \
                 tc.tile_pool(name=f"psM_{layer}", bufs=1,
                              space="PSUM") as psM, \
                 tc.tile_pool(name=f"psTD_{layer}", bufs=2,
                              space="PSUM") as psTD:
                ymlpT_ps = [psM.tile([P, T], F32, name=f"ymlpT_ps{k}",
                                     tag=f"ymlpT_ps{k}") for k in range(KD)]

                def emit_dec(m, c, xy):
                    for k in range(KD):
                        nc.tensor.matmul(
                            ymlpT_ps[k][:, c * 512:(c + 1) * 512],
                            lhsT=dec_sb[:, m, k * P:(k + 1) * P],
                            rhs=xy[:],
                            start=(m == 0), stop=(m == NM - 1))

                def epilogue_c(c):
                    """PSUM->SBUF copies, transposes, AR staging for half c"""
                    ymk = [wk16.tile([P, 512], HDT, name="ymk")
                           for _ in range(KD)]
                    for k in range(KD):
                        src = ymlpT_ps[k][:, c * 512:(c + 1) * 512]
                        if (k + c) % 2:
                            nc.scalar.copy(out=ymk[k][:], in_=src)
                        else:
                            nc.vector.tensor_copy(out=ymk[k][:], in_=src)
                    for j in range(4 * c, 4 * c + 4):
                        jo = (j - 4 * c) * P
                        for k in range(KD):
                            transpose_into(ymlp_nat[:, j, k * P:(k + 1) * P],
                                           ymk[k][:, jo:jo + P], psTD,
                                           nc.scalar if (j + k) % 2
                                           else nc.vector)
                    nc.scalar.dma_start(ar2_in_v[:, 4 * c:4 * c + 4, :],
                                        ymlp_nat[:, 4 * c:4 * c + 4, :])

                for c in range(2):
                    pend = []   # deferred dec matmuls (lag 2 for pipelining)
                    for m in range(NM):
                        ps = psD.tile([P, 512], F32, name="psD")
                        for k in range(KD):
                            nc.tensor.matmul(
                                ps[:],
                                lhsT=encv_sb[:, k, m * P:(m + 1) * P],
                                rhs=ykvT_h[:, k, c * 512:(c + 1) * 512],
                                start=(k == 0), stop=(k == KD - 1))
                        ysp = wk16.tile([P, 512], HDT, name="w1")
                        nc.scalar.activation(out=ysp[:], in_=ps[:],
                                             func=RELU)
                        xy = wk16.tile([P, 512], HDT, name="w3")
                        nc.vector.tensor_mul(
                            xy[:], x_sp[:, m // NPAIR, m % NPAIR,
                                        c * 512:(c + 1) * 512], ysp[:])
                        pend.append((m, c, xy))
                        if len(pend) >= 3:
                            emit_dec(*pend.pop(0))
                        if c == 1 and m == 8:
                            epilogue_c(0)
                    for m, c_, xy in pend:
                        emit_dec(m, c_, xy)
                epilogue_c(1)

            # === Phase E: 8-way AllReduce of ymlp; x = ln(x + ln(ymlp)) ===
            ar2_out = dram.tile([T, D], HDT, name=f"ar2out_{layer}",
                                tag=f"ar2out_{layer}", addr_space="Shared")
            emit_allreduce(nc, ALL_GROUP, [ar2_in.opt()], [ar2_out.opt()])
            ymlp_post = stg.tile([P, NT, D], HDT, name="stg")
            ar2_out_v = ar2_out.rearrange("(j p) d -> p j d", p=P)
            nc.sync.dma_start(ymlp_post[:, 0:4, :], ar2_out_v[:, 0:4, :])
            nc.sync.dma_start(ymlp_post[:, 4:8, :], ar2_out_v[:, 4:8, :])
            # next layer's encv/dec, gated past the AR window
            if prefetch_next:
                gate2 = statp.tile([P, 1], F32, name="gate")
                nc.vector.tensor_copy(out=gate2[:], in_=ymlp_post[:, 7, 0:1])
                encv_next = load_enc(1, gate2)
                dec_next = load_dec(gate2)
            last = layer == n_layers - 1
            if last:
                lmh_sb = wk16.tile([P, KD, VOCAB], HDT, name="w2", bufs=1)
                for k in range(KD):
                    nc.sync.dma_start(lmh_sb[:, k, :],
                                      lmh_d.ap()[k * P:(k + 1) * P, :])
            with tc.tile_pool(name=f"psE_{layer}", bufs=4,
                              space="PSUM") as psE, \
                 tc.tile_pool(name=f"psL_{layer}", bufs=2,
                              space="PSUM") as psL:
                for lo in (0, 4):   # halves: next layer's A starts on 0..3
                    ln_batch(ymlp_post, lambda j: ymlp_post[:, j, :],
                             lo, lo + 4)   # in-place
                    for j in range(lo, lo + 4):
                        nc.vector.tensor_add(ymlp_post[:, j, :],
                                             ymlp_post[:, j, :],
                                             x_h[:, j, :])
                    ln_batch(ymlp_post, lambda j: x_h[:, j, :], lo, lo + 4)
                    for j in range(lo, lo + 4):
                        for k in range(KD):
                            transpose_into(xT_h[:, k, j * P:(j + 1) * P],
                                           x_h[:, j, k * P:(k + 1) * P],
                                           psE,
                                           nc.scalar if (j + k) % 2
                                           else nc.vector)
                for j in range(NT):
                    if last:
                        # logits = x @ lm_head, fused into the final layer's
                        # epilogue per t-tile
                        ps = psL.tile([P, VOCAB], F32, name="psLt")
                        for k in range(KD):
                            nc.tensor.matmul(
                                ps[:], lhsT=xT_h[:, k, j * P:(j + 1) * P],
                                rhs=lmh_sb[:, k, :],
                                start=(k == 0), stop=(k == KD - 1))
                        lg = wk16.tile([P, VOCAB], F32, name="w1")
                        nc.scalar.copy(out=lg[:], in_=ps[:])
                        nc.sync.dma_start(
                            logits_d.ap()[j * P:(j + 1) * P, :], lg[:])

            if layer + 1 < n_layers:
                enc_sb, encv_sb, dec_sb = enc_next, encv_next, dec_next

        for _pool in (statp, stg, wk16, stp, ctp, decp,
                      encvp, encp, dram, persist):
            _pool.release()

    nc.compile()
    return nc


def _host_inputs(idx, embed, encoder, encoder_v, decoder, lm_head):
    """Build the 8 per-core input maps (host-side sharding)."""
    f16 = np.float16
    idx = np.asarray(idx).reshape(-1).astype(np.int64)
    embed = np.asarray(embed, np.float32)
    enc = np.asarray(encoder, np.float32)
    encv = np.asarray(encoder_v, np.float32)
    dec = np.asarray(decoder, np.float32)
    lmh = np.asarray(lm_head, np.float32)

    x0 = embed[idx]  # [T, D] gather on host (pure indexing)

    # freqs exactly as the reference computes them (fp32)
    t = np.arange(0, N, dtype=np.float32)
    q = np.floor(t / 2.0) * 2.0
    freqs = (1.0 / ((2.0 ** 16) ** (q / N)) / TWO_PI).astype(np.float32)
    tvec = np.arange(T, dtype=np.float32)

    umask = (np.arange(P)[:, None] < np.arange(P)[None, :]).astype(np.float32)

    in_maps = []
    for d in range(N_CORES):
        h, half = d // 2, d % 2
        perm = np.concatenate([np.arange(0, NLOC, 2),
                               np.arange(1, NLOC, 2)]) + half * NLOC
        f_loc = freqs[perm[:NLOC // 2]]
        ph = (tvec[None, :] * f_loc[:, None]).astype(np.float32) % 1.0
        in_maps.append({
            "x0": np.ascontiguousarray(x0, f16),
            "encw": np.ascontiguousarray(enc[h][:, perm], f16),
            "encvw": np.ascontiguousarray(encv[h][:, perm], f16),
            "decw": np.ascontiguousarray(dec[h * N + perm, :], f16),
            "ct": np.ascontiguousarray(np.cos(TWO_PI * ph), f16),
            "st": np.ascontiguousarray(np.sin(TWO_PI * ph), f16),
            "lmh": np.ascontiguousarray(lmh, f16),
            "umask": umask,
        })
    return in_maps


def kernel(idx, embed, encoder, encoder_v, decoder, lm_head,
           _trace=False, _tmpdir=None):
    if "nc" not in _CACHE:
        _CACHE["nc"] = _build_program()
    nc = _CACHE["nc"]
    in_maps = _host_inputs(idx, embed, encoder, encoder_v, decoder, lm_head)
    res = bass_utils.run_bass_kernel_spmd(
        nc, in_maps, core_ids=list(range(N_CORES)),
        trace=_trace, tmpdir=_tmpdir)
    _CACHE["last_results"] = res
    logits = res.results[0]["logits"].astype(np.float32).reshape(B, T, VOCAB)
    return logits
